# revision 1
# baseline (speedup 1.0000x reference)
"""CRF NLL (allpath - realpath) Trainium2 Bass kernel, 8-core data parallel.

Algorithm (per core, 128-batch slice):
  Forward-algorithm partition function and gold-path score are both computed
  in *scaled probability space*, so the per-step logsumexp-matvec becomes a
  real TensorEngine matmul with exp(transition) as the stationary operand.

  - Two sequential chains per core: forward (l=0..255) and backward
    (l=511..256, time-reversed on host) meet in the middle; this halves the
    sequential-dependency depth so the two chains' matmul/DVE ops interleave.
  - State tile S is (128, 128) bf16: partitions = 2 batch-groups x 64 tags
    (block-diagonal exp(transition) bf16 weights), free = [allpath p |
    goldpath w] x 64 batch lanes.  One matmul + one DVE multiply per step.
  - The gold-path chain w rides the same matmuls, multiplied by
    mt = 256 * [tag == gold] * exp(feat) instead of exp(feat).  The
    256*onehot(gold) mask ships from host as bf16; GPSIMD (otherwise idle)
    multiplies it with ACT's exp(feat) to form the masked half of in1.
  - exp(feat - 8*ln2) folds a 2^-8 per-step shrink into the ACT exp so state
    magnitudes drift slowly; every 64 steps a lazy power-of-2 renorm
    measures per-lane mass (PE) and exponent (DVE/GPSIMD bit tricks) off
    the critical path, then folds the 2^-e scale into the next chunk's
    step-4 in1 slice, so the recurrence never stalls.  Scaling exponents
    accumulate in int32; the final renorm rides the meet product.
  - Chunk prep (DMA + exp + mask-mult) is emitted two 32-step blocks ahead
    of use, its DVE sync-absorbers one block behind that, so the DVE queue
    never blocks on in-flight prep.  All bf16 constants ship as one DMA
    blob; the first chunk pair preps only 8 steps up front (gold-mult on
    DVE) so the recurrence starts ~10us into the kernel.

Host side only reorders/replicates input data (bf16 feats, bf16 one-hot
gold masks) and precomputes tiny constant tables (exp of the 64x64
transition matrix); all O(L*B*T) floating-point compute is on device.
"""
import os
import numpy as np
import ml_dtypes
from contextlib import ExitStack

BF16 = ml_dtypes.bfloat16

L, B, TAG = 512, 1024, 64
START, END = 62, 63
NCORE = 8
BC = B // NCORE          # 128 batch per core
CH = 32                  # steps per chunk
NCH = L // CH            # 16 chunks (8 fwd + 8 bwd)
HALF = L // 2            # 256 steps per direction
RENORM = 64              # renorm every this many steps
BIAS_BITS = 8.0          # fold 2^-8 per step into exp()
LN2 = float(np.log(2.0))

_CACHE = {}


def _emit(ctx, tc, nc, mybir, bass, dram):
    f32 = mybir.dt.float32
    i32 = mybir.dt.int32
    bf16 = mybir.dt.bfloat16
    AF = mybir.ActivationFunctionType
    OP = mybir.AluOpType

    fd, oh, cb, selbd, out_ext = dram

    consts = ctx.enter_context(tc.tile_pool(name="consts", bufs=1))
    fd_pool = ctx.enter_context(tc.tile_pool(name="fd", bufs=6))
    oh_pool = ctx.enter_context(tc.tile_pool(name="oh", bufs=6))
    in1_pool = ctx.enter_context(tc.tile_pool(name="in1", bufs=6))
    st_pool = ctx.enter_context(tc.tile_pool(name="state", bufs=6))
    sm_pool = ctx.enter_context(tc.tile_pool(name="small", bufs=8))
    sc_pool = ctx.enter_context(tc.tile_pool(name="sync", bufs=2))
    q_pool = ctx.enter_context(tc.tile_pool(name="qpsum", bufs=4, space="PSUM"))
    ax_pool = ctx.enter_context(tc.tile_pool(name="axpsum", bufs=4, space="PSUM"))

    # --- sync absorbers -------------------------------------------------
    # Each hardware instruction has ~2 sync-command slots (waits + update
    # combined), so any op that would wait on two other engines fails
    # codegen.  These 1-row dummy reads "absorb" a producer's semaphore
    # into the reading engine's observed clock; Tile then elides that wait
    # from every later op on the same engine.
    def dve_sync(ap_slice):
        t = sc_pool.tile([1, 128], f32, tag="dsync")
        nc.vector.tensor_copy(t[:, 0:ap_slice.shape[-1]], ap_slice)

    def act_sync(ap_slice):
        t = sc_pool.tile([1, 128], f32, tag="async")
        nc.scalar.copy(t[:, 0:ap_slice.shape[-1]], ap_slice)

    def pool_sync(ap_slice):
        t = sc_pool.tile([1, 128], f32, tag="psync")
        nc.gpsimd.tensor_copy(t[:, 0:ap_slice.shape[-1]], ap_slice)

    # --- constants ------------------------------------------------------
    # TensorEngine operands are bounced through a DVE copy so each matmul
    # waits only on the DVE proc.
    def mm_const(src, shape, tag, dt=f32):
        stage = sm_pool.tile(shape, dt, tag="cstage")
        nc.sync.dma_start(stage[:], src[:])
        t = consts.tile(shape, dt, tag=tag)
        nc.vector.tensor_copy(t[:], stage[:])
        return t

    # all 128-partition bf16 constants ride ONE dma + ONE bounce copy
    # (8 separate ~600ns DMA issues would gate the first chunk's fd)
    cstage = sm_pool.tile([128, 514], bf16, tag="cstage")
    cblob = consts.tile([128, 514], bf16, tag="cblob")

    def load_consts():
        nc.sync.dma_start(cstage[:], cb[:])
        nc.vector.tensor_copy(cblob[:], cstage[:])

    lf_t = cblob[:, 0:128]
    lb_t = cblob[:, 128:256]
    s0_t = cblob[:, 256:384]
    end_t = cblob[:, 384:512]
    ones_t = cblob[:, 512:514]
    sh23_t = consts.tile([2, 128], i32, tag="sh23")
    acc_t = consts.tile([2, 128], i32, tag="acc")
    sel_t = None

    def load_consts2():
        nc.vector.memset(sh23_t[:], 23)
        nc.vector.memset(acc_t[:], 0)
        return mm_const(selbd, [2, 128], "sel")

    # --- per-chunk prep -------------------------------------------------
    # in1 step block = [exp(feat) | exp(feat) * 256*onehot(gold)]: gold
    # lanes of the masked half are exactly 256*exp(feat), all others 0.
    # ACT exps the plain half, GPSIMD (otherwise idle) multiplies in the
    # host-built bf16 mask for the gold half.  The DVE-side sync absorbers
    # are deferred (returned as a closure) so the DVE queue doesn't block
    # on in-flight prep.
    def prep_chunk(ch, sf_cur, war_in1, dve_gold=False, split_k=None):
        fd_t = fd_pool.tile([128, CH * 64], bf16, tag="fd")
        oh_t = oh_pool.tile([128, CH * 64], bf16, tag="oh")
        nc.sync.dma_start(fd_t[:], fd[ch])
        nc.sync.dma_start(oh_t[:], oh[ch])
        if sf_cur is not None:
            act_sync(sf_cur[0:1, 0:1])         # absorb DVE (in1 slot WAR)
        act_sync(fd_t[0:1, 0:1])               # absorb fd DMA into ACT
        if war_in1 is not None:
            act_sync(war_in1[0:1, 64:65])      # absorb POOL (in1 slot WAR)
        in1_t = in1_pool.tile([128, CH * 128], bf16, tag="in1")
        in1_3d = in1_t.rearrange("p (k x) -> p k x", x=128)
        fd_3d = fd_t.rearrange("p (k x) -> p k x", x=64)
        oh_3d = oh_t.rearrange("p (k x) -> p k x", x=64)

        def piece_exp(k0, k1):
            nc.scalar.activation(in1_3d[:, k0:k1, 0:64], fd_3d[:, k0:k1, :],
                                 AF.Exp)

        def piece_gold(k0, k1):
            if dve_gold:
                # startup only: DVE does the gold multiply (Pool's serial
                # 4us tts would gate the first rounds otherwise)
                dve_sync(in1_t[0:1, k0 * 128:k0 * 128 + 64])
                nc.vector.tensor_tensor(in1_3d[:, k0:k1, 64:128],
                                        in1_3d[:, k0:k1, 0:64],
                                        oh_3d[:, k0:k1, :], OP.mult)
            else:
                pool_sync(in1_t[0:1, 0:1])     # absorb ACT into POOL
                if sf_cur is not None:
                    pool_sync(sf_cur[0:1, 0:1])  # absorb DVE (slot WAR)
                nc.gpsimd.tensor_tensor(in1_3d[:, k0:k1, 64:128],
                                        in1_3d[:, k0:k1, 0:64],
                                        oh_3d[:, k0:k1, :], OP.mult)

        if split_k is None:
            piece_exp(0, CH)
            piece_gold(0, CH)
            return in1_t

        piece_exp(0, split_k)
        piece_gold(0, split_k)
        return in1_t, piece_exp, piece_gold

    def prep_sync(in1_t):
        dve_sync(in1_t[0:1, 0:64])             # absorb ACT into DVE
        dve_sync(in1_t[0:1, 64:128])           # absorb POOL into DVE

    # --- renorm ---------------------------------------------------------
    # Off-critical-path renorm: PE sums per-lane mass, DVE extracts the
    # exponent, GPSIMD does the bookkeeping (acc += e, build 2^-e bits),
    # PE broadcasts 2^-e to a (128,128) tile.  The chain itself is only
    # touched by whoever multiplies sbc in: a future in1 slice (lazy) or
    # the state itself (final).
    def renorm_measure(s_t):
        mass = ax_pool.tile([2, 128], f32, tag="ax")
        nc.tensor.matmul(mass[:], ones_t[:], s_t[:], start=True, stop=True)
        eint = sm_pool.tile([2, 128], i32, tag="eint")
        nc.vector.tensor_tensor(eint[:], mass.bitcast(i32)[:], sh23_t[:],
                                OP.logical_shift_right)
        nc.gpsimd.tensor_tensor(acc_t[:], acc_t[:], eint[:], OP.add)
        sbits = sm_pool.tile([2, 128], i32, tag="sbits")
        nc.gpsimd.tensor_scalar(sbits[:], eint[:], -(1 << 23), 0x7F000000,
                                OP.mult, OP.add)
        return sbits

    def make_sbc(sbits):
        # deferred: by now sbits is long done, so this never blocks PE
        sbc = ax_pool.tile([128, 128], f32, tag="ax")
        nc.tensor.matmul(sbc[:], sel_t[:], sbits.bitcast(f32)[:],
                         start=True, stop=True)
        return sbc

    # --- interleaved fwd/bwd chains, 32-step blocks ---------------------
    # Chunk prep is emitted two blocks ahead of use; its DVE sync
    # absorbers one block ahead.
    sf = s0_t
    sb = None
    prep_hist = []
    prepped = {}

    def prep_pair(b, sf_cur):
        old = prep_hist[-4] if len(prep_hist) >= 4 else None
        old2 = prep_hist[-3] if len(prep_hist) >= 4 else None
        first = len(prep_hist) == 0
        if first:
            # pair 0: only the first 8 steps of exp/gold-mult (on DVE) are
            # emitted now, so the chain's first rounds start ~13us earlier;
            # the rest-exps go out before pair 1's (ACT is in-order), the
            # rest-gold-mults mid-block via the closures
            x, ex, gx = prep_chunk(b, None, None, dve_gold=True, split_k=8)
            y, ey, gy = prep_chunk(8 + b, None, None, dve_gold=True,
                                   split_k=8)
            ex(8, CH)
            ey(8, CH)
            prep_hist.extend([x, y])
            prepped[b] = (x, y)
            return {4: [lambda: gx(8, CH), lambda: gy(8, CH)]}
        x = prep_chunk(b, sf_cur, old)
        y = prep_chunk(8 + b, sf_cur, old2)
        prep_hist.extend([x, y])
        prepped[b] = (x, y)
        return []

    load_consts()
    rest0 = prep_pair(0, None)
    sel_t = load_consts2()
    prep_pair(1, s0_t)
    pending = []
    for blk in range(8):
        in1_f, in1_b = prepped.pop(blk)
        in1f_3d = in1_f.rearrange("p (k x) -> p k x", x=128)
        in1b_3d = in1_b.rearrange("p (k x) -> p k x", x=128)
        if blk + 2 <= 7:
            prep_pair(blk + 2, sf)
        for k in range(CH):
            step = blk * CH + k
            if k == 4 and pending:
                # apply pending lazy renorm scales into this chunk's step-4
                # in1 slice: by round 4 the GPSIMD exponent bookkeeping is
                # long done, so the sbc matmuls never stall the PE queue
                for tgt, sbits in pending:
                    sbc = make_sbc(sbits)
                    nc.vector.tensor_mul(tgt[:, 4, :], tgt[:, 4, :], sbc[:])
                pending = []
            if blk == 0 and k in rest0:
                # emit the rest of pair 0's gold-mults piecewise; each
                # piece's exp input is ready by its emission point
                for r in rest0[k]:
                    r()
            qf = q_pool.tile([128, 128], f32, tag="q")
            nc.tensor.matmul(qf[:], lf_t[:], sf[:], start=True, stop=True)
            sf_new = st_pool.tile([128, 128], bf16, tag="st")
            nc.vector.tensor_mul(sf_new[:], qf[:], in1f_3d[:, k, :])
            sf = sf_new
            if blk == 0 and k == 0:
                sb = st_pool.tile([128, 128], bf16, tag="st")
                nc.vector.tensor_tensor(sb[:], in1b_3d[:, 0, :], end_t[:],
                                        OP.mult)
            else:
                qb = q_pool.tile([128, 128], f32, tag="q")
                nc.tensor.matmul(qb[:], lb_t[:], sb[:], start=True,
                                 stop=True)
                sb_new = st_pool.tile([128, 128], bf16, tag="st")
                nc.vector.tensor_mul(sb_new[:], qb[:], in1b_3d[:, k, :])
                sb = sb_new
            if (step + 1) % RENORM == 0:
                if step + 1 == HALF:
                    final_sbits = (renorm_measure(sf), renorm_measure(sb))
                else:
                    nf, nb = prepped[blk + 1]
                    nf3 = nf.rearrange("p (k x) -> p k x", x=128)
                    nb3 = nb.rearrange("p (k x) -> p k x", x=128)
                    pending = [(nf3, renorm_measure(sf)),
                               (nb3, renorm_measure(sb))]
        # absorb the *next* chunk pair's prep into DVE at end-of-block:
        # by now its ACT exp and POOL gold-mult have had a full block to
        # finish, so these never stall the queue
        if blk + 1 <= 7:
            prep_sync(prepped[blk + 1][0])
            prep_sync(prepped[blk + 1][1])
        if blk == 6:
            # preload the Ln table while ACT is idle so the extraction's
            # Ln pays no drain + table load; reading pair 7's in1 pins
            # this AFTER the last Exp, so the scheduler can't hoist it
            lnwarm = sm_pool.tile([1, 2], f32, tag="lnwarm")
            nc.scalar.activation(lnwarm[:], prepped[7][0][0:1, 0:2], AF.Ln)

    # --- meet in the middle & extraction --------------------------------
    # the final renorm's scales ride the meet product instead of the state
    v = q_pool.tile([128, 128], f32, tag="q")
    nc.tensor.matmul(v[:], lb_t[:], sb[:], start=True, stop=True)
    sbc_f = make_sbc(final_sbits[0])
    sbc_b = make_sbc(final_sbits[1])
    dve_sync(v[0:1, 0:1])
    p2 = st_pool.tile([128, 128], bf16, tag="st")
    nc.vector.tensor_mul(p2[:], v[:], sf[:])
    p3 = st_pool.tile([128, 128], bf16, tag="st")
    nc.vector.tensor_mul(p3[:], sbc_f[:], p2[:])
    p4 = st_pool.tile([128, 128], bf16, tag="st")
    nc.vector.tensor_mul(p4[:], sbc_b[:], p3[:])
    meet = ax_pool.tile([2, 128], f32, tag="ax")
    nc.tensor.matmul(meet[:], ones_t[:], p4[:], start=True, stop=True)
    act_sync(meet[0:1, 0:1])                   # absorb PE into ACT
    lnm = sm_pool.tile([2, 128], f32, tag="lnm")
    nc.scalar.activation(lnm[:], meet[:], AF.Ln)
    dve_sync(lnm[0:1, 0:1])                    # absorb ACT into DVE
    dve_sync(acc_t[0:1, 0:1])                  # absorb POOL (acc) into DVE
    # answer = lnA - lnR + (accA - accR + 8*L) * ln2
    dacc = sm_pool.tile([2, 64], i32, tag="dacc")
    nc.vector.tensor_sub(dacc[:], acc_t[:, 0:64], acc_t[:, 64:128])
    daccf = sm_pool.tile([2, 64], f32, tag="daccf")
    nc.vector.tensor_copy(daccf[:], dacc[:])
    t1 = sm_pool.tile([2, 64], f32, tag="t1")
    nc.vector.tensor_sub(t1[:], lnm[:, 0:64], lnm[:, 64:128])
    t2 = sm_pool.tile([2, 64], f32, tag="t2")
    nc.vector.tensor_scalar(t2[:], daccf[:], LN2, BIAS_BITS * L * LN2,
                            OP.mult, OP.add)
    ans = sm_pool.tile([2, 64], f32, tag="ans")
    nc.vector.tensor_add(ans[:], t1[:], t2[:])
    nc.sync.dma_start(out_ext.rearrange("(p x) -> p x", p=2), ans[:])


def build():
    if "nc" in _CACHE:
        return _CACHE["nc"]
    import concourse.bass as bass
    import concourse.tile as tile
    from concourse import bacc, mybir

    f32 = mybir.dt.float32
    bf16 = mybir.dt.bfloat16
    nc = bacc.Bacc("TRN2", debug=False)
    nc.all_engine_barrier()
    fd = nc.dram_tensor("fd", [NCH, 128, CH * 64], bf16, kind="ExternalInput").ap()
    oh = nc.dram_tensor("oh", [NCH, 128, CH * 64], bf16, kind="ExternalInput").ap()
    cb = nc.dram_tensor("cb", [128, 514], bf16, kind="ExternalInput").ap()
    selbd = nc.dram_tensor("selbd", [2, 128], f32, kind="ExternalInput").ap()
    out_ext = nc.dram_tensor("out", [BC], f32, kind="ExternalOutput").ap()
    dram = (fd, oh, cb, selbd, out_ext)
    with ExitStack() as ctx:
        tc = ctx.enter_context(tile.TileContext(nc))
        _emit(ctx, tc, nc, mybir, bass, dram)
    nc.compile()
    _CACHE["nc"] = nc
    return nc


def host_prepare(feats, tags, transition):
    """Vectorized host-side data arrangement for all 8 cores."""
    feats = np.asarray(feats, dtype=np.float32)
    tags = np.asarray(tags)
    transition = np.asarray(transition, dtype=np.float32)

    # FD[c, ch, p=(g,t), k, b0] = feats[l(ch,k), 128c + 64g + b0, t]
    ft = feats.reshape(L, NCORE, 2, 64, TAG).transpose(1, 0, 2, 4, 3)
    ft = ft.reshape(NCORE, L, 128, 64)                    # (c, l, p, b0)
    fwd = ft[:, :HALF].reshape(NCORE, 8, CH, 128, 64).transpose(0, 1, 3, 2, 4)
    bwd = ft[:, HALF:][:, ::-1].reshape(NCORE, 8, CH, 128, 64)
    bwd = bwd.transpose(0, 1, 3, 2, 4)
    FD = np.concatenate([fwd, bwd], axis=1)               # (c, 16, 128, 32, 64)
    FD = np.ascontiguousarray(FD).reshape(NCORE, NCH, 128, CH * 64)

    # 256 * onehot(gold tag) in the same (c, ch, p, k*64) layout, bf16.
    # partition p = (g, t) carries tag t of batch group g; gold lane hits
    # where t == tags[l, b] for that group's batch lane.
    tg = tags.astype(np.int16).reshape(L, NCORE, 2, 64).transpose(1, 0, 2, 3)
    tg = tg + (np.arange(2, dtype=np.int16) * 64)[None, None, :, None]
    tgf = tg[:, :HALF].reshape(NCORE, 8, CH, 2, 64)
    tgb = tg[:, HALF:][:, ::-1].reshape(NCORE, 8, CH, 2, 64)
    t6 = np.concatenate([tgf, tgb], axis=1)               # (c, ch, k, g, b0)
    part = np.arange(128, dtype=np.int16)
    OH = (t6[:, :, None, :, :, :] == part[None, None, :, None, None, None])
    # (c, ch, p, k, g, b0): partition p already encodes g via the +64
    # offset, so collapsing g keeps exactly the matching group's lanes
    OH = OH.any(axis=4)                                   # (c, ch, p, k, b0)
    OH = (OH.astype(np.float32) * 256.0).astype(BF16)
    OH = np.ascontiguousarray(OH).reshape(NCORE, NCH, 128, CH * 64)

    E = (np.exp(transition) * 2.0 ** -BIAS_BITS).astype(np.float32)
    lf = np.zeros((128, 128), np.float32)
    lb = np.zeros((128, 128), np.float32)
    for g in range(2):
        s = slice(64 * g, 64 * g + 64)
        lf[s, s] = E.T
        lb[s, s] = E
    onesbd = np.zeros((128, 2), np.float32)
    onesbd[0:64, 0] = 1.0
    onesbd[64:128, 1] = 1.0
    selbd = np.zeros((2, 128), np.float32)
    selbd[0, 0:64] = 1.0
    selbd[1, 64:128] = 1.0
    endbc = np.tile(np.exp(transition[END, :]).astype(np.float32), 2)
    endbc = np.repeat(endbc.reshape(128, 1), 128, axis=1)
    s0 = np.zeros((128, 128), np.float32)
    s0[START, :] = 1.0
    s0[64 + START, :] = 1.0
    cb = np.concatenate([lf, lb, s0, endbc, onesbd], axis=1)  # (128, 514)
    return FD.astype(BF16), OH, cb.astype(BF16), selbd


def _install_ntff_hook():
    """Provide antenv.axon_hooks (absent in this image) so trace=True can
    capture NTFF profiles via the axon .so C ABI."""
    import sys, types, ctypes, contextlib
    if "antenv.axon_hooks" in sys.modules:
        return
    so_path = None
    for line in open("/proc/self/maps"):
        if "libaxon_pjrt.so" in line:
            so_path = line.split()[-1]
            break
    mod = types.ModuleType("antenv.axon_hooks")
    state = {"hook": None}
    if so_path:
        lib = ctypes.CDLL(so_path)
        if hasattr(lib, "axon_start_nrt_profile"):
            lib.axon_start_nrt_profile.argtypes = [
                ctypes.POINTER(ctypes.c_int64), ctypes.c_size_t]
            lib.axon_start_nrt_profile.restype = ctypes.c_int64
            lib.axon_stop_nrt_profile.argtypes = [ctypes.c_char_p]
            lib.axon_stop_nrt_profile.restype = ctypes.c_int64

            @contextlib.contextmanager
            def _hook(output_dir, device_ids):
                import jax
                jax.devices()
                if device_ids:
                    ids = (ctypes.c_int64 * len(device_ids))(*device_ids)
                    rc = lib.axon_start_nrt_profile(ids, len(device_ids))
                else:
                    rc = lib.axon_start_nrt_profile(None, 0)
                if rc != 0:
                    raise RuntimeError(f"axon_start_nrt_profile rc={rc}")
                try:
                    yield
                finally:
                    n = lib.axon_stop_nrt_profile(str(output_dir).encode())
                    print(f"ntff profile: {n} file(s) -> {output_dir}")

            state["hook"] = _hook
    mod.get_axon_ntff_profile_hook = lambda: state["hook"]
    mod.set_axon_ntff_profile_hook = lambda h: state.update(hook=h)
    sys.modules["antenv.axon_hooks"] = mod


def kernel(feats, tags, mask, transition):
    from concourse.bass_utils import run_bass_kernel_spmd
    if os.environ.get("CRF_TRACE", "0") == "1":
        _install_ntff_hook()

    tags_np = np.asarray(tags)
    FD, OH, cb, selbd = host_prepare(feats, tags_np, transition)
    nc = build()
    in_maps = []
    for c in range(NCORE):
        in_maps.append({"fd": FD[c], "oh": OH[c], "cb": cb, "selbd": selbd})
    res = run_bass_kernel_spmd(nc, in_maps, list(range(NCORE)),
                               trace=bool(int(os.environ.get("CRF_TRACE", "0"))))
    out = np.concatenate([np.asarray(res.results[c]["out"]).reshape(BC)
                          for c in range(NCORE)])
    if getattr(res, "exec_time_ns", None):
        print(f"HW exec time: {res.exec_time_ns} ns")
    return out.astype(np.float32)



# revision 14
# speedup vs baseline: 1.6162x; 1.6162x over previous
"""CRF NLL (allpath - realpath) Trainium2 Bass kernel, 8-core data parallel.

v2: segmented forward algorithm.  The sequential depth of the forward
recurrence is cut 4x by splitting the 512-step chain into S=8 segments of
64 steps.  Products of 64 positive transfer matrices are numerically rank-1
(Perron-Frobenius contraction; measured error ~1e-12 in fp64), so each
interior segment product P_s is replaced by (P_s g)(h^T P_s)/(h^T P_s g)
with probe vectors g = h = ones.  This yields 7 forward chains and 7
backward chains, all independent, each 64 steps deep:

  Z = e^T P_7 ... P_0 s0
    ~= (w^T a_6)(b_6^T a_5)...(b_1^T a_0) / (c_1 ... c_6)

where a_s = P_s g (a_0 uses the true s0), b_s^T = h^T P_s (b_7 uses the
true e = exp(trans[END])), and c_s = h^T a_s (the fwd-chain mass, free).

Chains run in scaled probability space: each step is one TensorEngine
matmul (stationary block-diag W = diag(E^T, E), E = exp(transition)*2^-8)
plus one DVE multiply with exp(feat).  Chains are packed in pairs
[fwd seg s ; bwd seg s+1] stacked on the 128 partitions, so every
inter-segment dot b_{s+1}^T a_s is a column-aligned top-half/bottom-half
product inside one tile.  The 7 pairs ride in two wide tiles (FD=512 and
FD=384); per round the engines see 2 matmuls + 2 DVE multiplies for all
14 chains, amortizing the DVE per-instruction fixed cost.

One power-of-2 renorm per chain at round 31 (mass via ones-matmul,
exponent extract via bit tricks on DVE/GPSIMD, scale folded lazily into a
future exp(feat) slice) keeps bf16 in range; exponents accumulate in int32
and fold into the final log.  Interior fwd-chain exponents cancel against
the c_s terms and are not tracked.

The gold-path score is a pure gather (no FP): the host ships
feats[l,b,tags[l,b]], transition[tag_{l+1},tag_l] and the END term as a
bf16 blob; the device reduces it with 9 accumulating ones-matmuls.
Host-side work is only data rearrangement/gather plus tiny O(T^2)
constant tables, as in v1; all O(L*B*T) floating-point math is on device.
"""
import os
import numpy as np
import ml_dtypes
from contextlib import ExitStack

BF16 = ml_dtypes.bfloat16

L, B, TAG = 512, 1024, 64
START, END = 62, 63
NCORE = 8
S = 8                   # segments
SEG = L // S            # 64 rounds
NPAIR = S - 1           # 7 chain pairs
FD1, FD2 = 512, 384     # tile1 = pairs 0-3, tile2 = pairs 4-6
COLS = NPAIR * 128      # 896
BIAS_BITS = 8.0
RENORM_AT = SEG // 2 - 1            # measure at round 31
APPLY_AT = RENORM_AT + 5            # fold scale into in1 of round 36
LN2 = float(np.log(2.0))
CHS = [(0, 2), (2, 4), (4, 8), (8, 12), (12, 16), (16, 24), (24, 32),
       (32, 40), (40, 48), (48, 56), (56, 64)]
NBUF_IN1 = 4

_CACHE = {}


def _emit(ctx, tc, nc, mybir, bass, dram):
    f32 = mybir.dt.float32
    i32 = mybir.dt.int32
    bf16 = mybir.dt.bfloat16
    AF = mybir.ActivationFunctionType
    OP = mybir.AluOpType

    fd, cbd, seld, goldd, out_ext = dram

    consts = ctx.enter_context(tc.tile_pool(name="consts", bufs=1))
    fd_pool = ctx.enter_context(tc.tile_pool(name="fd", bufs=2))
    in1_pool = ctx.enter_context(tc.tile_pool(name="in1", bufs=NBUF_IN1))
    st_pool = ctx.enter_context(tc.tile_pool(name="state", bufs=4))
    sm_pool = ctx.enter_context(tc.tile_pool(name="small", bufs=1))
    sc_pool = ctx.enter_context(tc.tile_pool(name="sync", bufs=2))
    q_pool = ctx.enter_context(tc.tile_pool(name="qpsum", bufs=2, space="PSUM"))
    ax_pool = ctx.enter_context(tc.tile_pool(name="axpsum", bufs=2, space="PSUM"))

    # --- sync absorbers (see v1): a 1-row dummy read makes the reading
    # engine's clock observe a producer's semaphore so Tile can elide that
    # wait from later ops on the same engine.
    def dve_sync(ap_slice):
        t = sc_pool.tile([1, 128], f32, tag="dsync")
        nc.vector.tensor_copy(t[:, 0:ap_slice.shape[-1]], ap_slice)

    def act_sync(ap_slice):
        t = sc_pool.tile([1, 128], f32, tag="async")
        nc.scalar.copy(t[:, 0:ap_slice.shape[-1]], ap_slice)

    def pool_sync(ap_slice):
        t = sc_pool.tile([1, 128], f32, tag="psync")
        nc.gpsimd.tensor_copy(t[:, 0:ap_slice.shape[-1]], ap_slice)

    # --- constants -------------------------------------------------------
    CBW = 128 + 128 + COLS + 2 + 1      # W | Wfin | init | onesbd | onesfull
    cstage = sm_pool.tile([128, CBW], bf16, tag="cstage")
    cblob = consts.tile([128, CBW], bf16, tag="cblob")
    nc.sync.dma_start(cstage[:], cbd[:])
    nc.vector.tensor_copy(cblob[:], cstage[:])
    W_t = cblob[:, 0:128]
    Wfin_t = cblob[:, 128:256]
    init_t = cblob[:, 256:256 + COLS]
    onesbd_t = cblob[:, 256 + COLS:258 + COLS]
    ones64_t = cblob[64:128, 257 + COLS:258 + COLS]   # onesbd col 1, bottom half
    onesfull_t = cblob[:, 258 + COLS:259 + COLS]

    selstage = sm_pool.tile([2, 130], f32, tag="selstage")
    sel_t = consts.tile([2, 130], f32, tag="sel")
    nc.sync.dma_start(selstage[:], seld[:])
    nc.vector.tensor_copy(sel_t[:], selstage[:])
    ones2_t = sel_t[:, 128:129]

    goldsb = consts.tile([128, 9 * 128], bf16, tag="goldsb")
    nc.sync.dma_start(goldsb[:], goldd[:])
    gold3d = goldsb.rearrange("p (g x) -> p g x", x=128)

    acc_t = consts.tile([2, COLS], i32, tag="acc")
    sh23_t = consts.tile([2, COLS], i32, tag="sh23")
    nc.vector.memset(acc_t[:], 0)
    nc.vector.memset(sh23_t[:], 23)

    # --- per-chunk prep --------------------------------------------------
    preps = {}

    def prep(ci, sf_cur):
        lo, hi = CHS[ci]
        n = hi - lo
        fd_t = fd_pool.tile([128, n * COLS], bf16, tag="fd")
        nc.sync.dma_start(fd_t[:], fd[:, lo * COLS:hi * COLS])
        if sf_cur is not None:
            act_sync(sf_cur[0:1, 0:1])     # absorb DVE (in1 buffer WAR)
        act_sync(fd_t[0:1, 0:1])           # absorb fd DMA into ACT
        in1_t = in1_pool.tile([128, n * COLS], bf16, tag="in1")
        in1_3d = in1_t.rearrange("p (k x) -> p k x", x=COLS)
        fd_3d = fd_t.rearrange("p (k x) -> p k x", x=COLS)
        nc.scalar.activation(in1_3d[:, :, :], fd_3d[:, :, :], AF.Exp)
        preps[ci] = in1_t
        return in1_t

    def prep_sync(in1_t):
        dve_sync(in1_t[0:1, 0:1])          # absorb ACT into DVE

    # --- startup ---------------------------------------------------------
    prep(0, None)
    prep(1, None)
    prep(2, None)
    dve_sync(cstage[0:1, 0:1])             # (cstage DMA already DVE-bounced)
    prep_sync(preps[0])

    in1c = preps[0]
    in1c_3d = in1c.rearrange("p (k x) -> p k x", x=COLS)
    sA = st_pool.tile([128, FD1], bf16, tag="stA")
    nc.vector.tensor_tensor(sA[:], init_t[:, 0:FD1], in1c_3d[:, 0, 0:FD1],
                            OP.mult)
    sB = st_pool.tile([128, FD2], bf16, tag="stB")
    nc.vector.tensor_tensor(sB[:], init_t[:, FD1:COLS], in1c_3d[:, 0, FD1:COLS],
                            OP.mult)

    emit_at = {CHS[1][0]: 3, CHS[2][0]: 4, CHS[3][0]: 5, CHS[4][0]: 6,
               CHS[5][0]: 7, CHS[6][0]: 8, CHS[7][0]: 9, CHS[8][0]: 10}
    ci = 0
    renorm_state = {}
    for r in range(1, SEG):
        if r in emit_at:
            prep(emit_at[r], sA)
        lo, hi = CHS[ci]
        if r >= hi:
            ci += 1
            lo, hi = CHS[ci]
            prep_sync(preps[ci])       # absorb this chunk's exp into DVE
            in1c = preps.pop(ci)
            in1c_3d = in1c.rearrange("p (k x) -> p k x", x=COLS)
        k = r - lo

        qA = q_pool.tile([128, FD1], f32, tag="q1")
        nc.tensor.matmul(qA[:], W_t, sA[:], start=True, stop=True)
        sA_new = st_pool.tile([128, FD1], bf16, tag="stA")
        nc.vector.tensor_mul(sA_new[:], qA[:], in1c_3d[:, k, 0:FD1])
        sA = sA_new

        qB = q_pool.tile([128, FD2], f32, tag="q2")
        nc.tensor.matmul(qB[:], W_t, sB[:], start=True, stop=True)
        sB_new = st_pool.tile([128, FD2], bf16, tag="stB")
        nc.vector.tensor_mul(sB_new[:], qB[:], in1c_3d[:, k, FD1:COLS])
        sB = sB_new

        if r == RENORM_AT:
            massA = ax_pool.tile([2, FD1], f32, tag="axA")
            nc.tensor.matmul(massA[:], onesbd_t, sA[:], start=True, stop=True)
            massB = ax_pool.tile([2, FD2], f32, tag="axB")
            nc.tensor.matmul(massB[:], onesbd_t, sB[:], start=True, stop=True)
            renorm_state["mass"] = (massA, massB)
        elif r == RENORM_AT + 2:
            massA, massB = renorm_state.pop("mass")
            eintA = sm_pool.tile([2, FD1], i32, tag="eintA")
            nc.vector.tensor_tensor(eintA[:], massA.bitcast(i32)[:],
                                    sh23_t[:, 0:FD1], OP.logical_shift_right)
            eintB = sm_pool.tile([2, FD2], i32, tag="eintB")
            nc.vector.tensor_tensor(eintB[:], massB.bitcast(i32)[:],
                                    sh23_t[:, 0:FD2], OP.logical_shift_right)
            pool_sync(eintB[0:1, 0:1])     # absorb DVE into GPSIMD
            sbitsA = sm_pool.tile([2, FD1], i32, tag="sbitsA")
            nc.gpsimd.tensor_scalar(sbitsA[:], eintA[:], -(1 << 23),
                                    0x7F000000, OP.mult, OP.add)
            sbitsB = sm_pool.tile([2, FD2], i32, tag="sbitsB")
            nc.gpsimd.tensor_scalar(sbitsB[:], eintB[:], -(1 << 23),
                                    0x7F000000, OP.mult, OP.add)
            # acc tracks pair-0 fwd (exact) + all bwd rows only; interior fwd
            # exponents cancel against the c_s mass terms.  Row-1-only slices
            # are illegal (partition base must be 32-aligned), so zero the
            # unwanted row-0 entries and accumulate full 2-row tiles.
            nc.vector.memset(eintA[0:1, 128:FD1], 0)
            nc.vector.memset(eintB[0:1, 0:FD2], 0)
            pool_sync(eintB[0:1, 0:1])     # absorb the memsets into GPSIMD
            nc.gpsimd.tensor_tensor(acc_t[:, 0:FD1], acc_t[:, 0:FD1],
                                    eintA[:], OP.add)
            nc.gpsimd.tensor_tensor(acc_t[:, FD1:COLS], acc_t[:, FD1:COLS],
                                    eintB[:], OP.add)
            renorm_state["sbits"] = (sbitsA, sbitsB)
        elif r == APPLY_AT - 2:
            sbitsA, sbitsB = renorm_state.pop("sbits")
            sbcA = q_pool.tile([128, FD1], f32, tag="q1")
            nc.tensor.matmul(sbcA[:], sel_t[:, 0:128], sbitsA.bitcast(f32)[:],
                             start=True, stop=True)
            sbcB = q_pool.tile([128, FD2], f32, tag="q2")
            nc.tensor.matmul(sbcB[:], sel_t[:, 0:128], sbitsB.bitcast(f32)[:],
                             start=True, stop=True)
            renorm_state["sbc"] = (sbcA, sbcB)
        elif r == APPLY_AT - 1:
            # fold 2^-e into the NEXT round's in1 slice; emitted here so it
            # precedes round APPLY_AT's chain TTs on the DVE queue
            sbcA, sbcB = renorm_state.pop("sbc")
            tci = _chunk_of(APPLY_AT)
            tgt = preps[tci] if tci in preps else in1c
            tgt3d = tgt.rearrange("p (k x) -> p k x", x=COLS)
            kk = APPLY_AT - CHS[tci][0]
            nc.vector.tensor_mul(tgt3d[:, kk, 0:FD1], tgt3d[:, kk, 0:FD1],
                                 sbcA[:])
            nc.vector.tensor_mul(tgt3d[:, kk, FD1:COLS], tgt3d[:, kk, FD1:COLS],
                                 sbcB[:])

    # warm the Ln table while ACT is idle (ACT is in-order, so this lands
    # after the final Exp and the table switch is paid before the tail)
    lnwarm = sm_pool.tile([1, 2], f32, tag="lnwarm")
    nc.scalar.activation(lnwarm[:], in1c[0:1, 0:2], AF.Ln)

    # --- tail: meets, masses, gold, combine ------------------------------
    q2A = q_pool.tile([128, FD1], f32, tag="q1")
    nc.tensor.matmul(q2A[:], Wfin_t, sA[:], start=True, stop=True)
    q2B = q_pool.tile([128, FD2], f32, tag="q2")
    nc.tensor.matmul(q2B[:], Wfin_t, sB[:], start=True, stop=True)
    pA = st_pool.tile([128, FD1], bf16, tag="pA")
    nc.vector.tensor_mul(pA[64:128, :], q2A[64:128, :], sA[64:128, :])
    pB = st_pool.tile([128, FD2], bf16, tag="pB")
    nc.vector.tensor_mul(pB[64:128, :], q2B[64:128, :], sB[64:128, :])
    dve_sync(q2B[0:1, 0:1])

    meetA = ax_pool.tile([2, FD1], f32, tag="axA")
    nc.tensor.matmul(meetA[0:1, :], ones64_t, pA[64:128, :], start=True,
                     stop=True)
    meetB = ax_pool.tile([2, FD2], f32, tag="axB")
    nc.tensor.matmul(meetB[0:1, :], ones64_t, pB[64:128, :], start=True,
                     stop=True)
    cmassA = ax_pool.tile([2, FD1], f32, tag="axA")
    nc.tensor.matmul(cmassA[:], onesbd_t, sA[:], start=True, stop=True)
    cmassB = ax_pool.tile([2, FD2], f32, tag="axB")
    nc.tensor.matmul(cmassB[:], onesbd_t, sB[:], start=True, stop=True)

    act_sync(cmassB[0:1, 0:1])             # absorb PE into ACT
    lnmA = sm_pool.tile([1, FD1], f32, tag="lnmA")
    nc.scalar.activation(lnmA[:], meetA[0:1, :], AF.Ln)
    lnmB = sm_pool.tile([1, FD2], f32, tag="lnmB")
    nc.scalar.activation(lnmB[:], meetB[0:1, :], AF.Ln)
    lnCA = sm_pool.tile([1, FD2], f32, tag="lnCA")   # pairs 1-3
    nc.scalar.activation(lnCA[:], cmassA[0:1, 128:FD1], AF.Ln)
    lnCB = sm_pool.tile([1, FD2], f32, tag="lnCB")   # pairs 4-6
    nc.scalar.activation(lnCB[:], cmassB[0:1, :], AF.Ln)

    # gold reduction reuses the axB ring (its mass/meet tiles are consumed
    # by the Lns above; Tile's WAR waits order the overwrite correctly)
    goldt = ax_pool.tile([2, FD2], f32, tag="axB", name="goldt")
    goldp = goldt[0:1, 0:128]
    for g in range(9):
        nc.tensor.matmul(goldp, onesfull_t, gold3d[:, g, :],
                         start=(g == 0), stop=(g == 8))

    # accsum[0, col] = acc[0, col] + acc[1, col]; reuse the axA/axB PSUM
    # rings (the mass/meet tiles they held are consumed by the Lns above,
    # so Tile's WAR waits order these correctly)
    accf = sm_pool.tile([2, COLS], f32, tag="accf")
    nc.vector.tensor_copy(accf[:], acc_t[:])
    accs1 = ax_pool.tile([2, FD1], f32, tag="axA")
    accs2 = ax_pool.tile([2, FD2], f32, tag="axB")
    # fp32 moving operand is capped at 512 columns per matmul
    nc.tensor.matmul(accs1[0:1, :], ones2_t, accf[:, 0:FD1],
                     start=True, stop=True)
    nc.tensor.matmul(accs2[0:1, :], ones2_t, accf[:, FD1:COLS],
                     start=True, stop=True)

    dve_sync(lnCB[0:1, 0:1])               # absorb ACT into DVE
    dve_sync(accs2[0:1, 0:1])              # absorb PE into DVE

    # small f32 scratch tiles
    def scratch(tag, n=128):
        return sm_pool.tile([1, n], f32, tag=tag, name=tag)

    m1 = scratch("m1", 256)
    nc.vector.tensor_add(m1[:], lnmA[0:1, 0:256], lnmA[0:1, 256:512])
    m2 = scratch("m2")
    nc.vector.tensor_add(m2[:], m1[0:1, 0:128], m1[0:1, 128:256])
    m3 = scratch("m3")
    nc.vector.tensor_add(m3[:], lnmB[0:1, 0:128], lnmB[0:1, 128:256])
    m4 = scratch("m4")
    nc.vector.tensor_add(m4[:], m3[:], lnmB[0:1, 256:384])
    m5 = scratch("m5")
    nc.vector.tensor_add(m5[:], m2[:], m4[:])
    c1 = scratch("c1")
    nc.vector.tensor_add(c1[:], lnCA[0:1, 0:128], lnCA[0:1, 128:256])
    c2 = scratch("c2")
    nc.vector.tensor_add(c2[:], c1[:], lnCA[0:1, 256:384])
    c3 = scratch("c3")
    nc.vector.tensor_add(c3[:], lnCB[0:1, 0:128], lnCB[0:1, 128:256])
    c4 = scratch("c4")
    nc.vector.tensor_add(c4[:], c3[:], lnCB[0:1, 256:384])
    c5 = scratch("c5")
    nc.vector.tensor_add(c5[:], c2[:], c4[:])
    t6 = scratch("t6")
    nc.vector.tensor_sub(t6[:], m5[:], c5[:])
    # acc per lane j: sum of accs over the 7 pair column-blocks (bounce the
    # PSUM rows to SBUF first: DVE allows only one PSUM input per op)
    as1 = scratch("as1", FD1)
    nc.vector.tensor_copy(as1[:], accs1[0:1, :])
    as2 = scratch("as2", FD2)
    nc.vector.tensor_copy(as2[:], accs2[0:1, :])
    a1 = scratch("a1", 256)
    nc.vector.tensor_add(a1[:], as1[0:1, 0:256], as1[0:1, 256:512])
    a2 = scratch("a2", 256)
    nc.vector.tensor_add(a2[:], as2[0:1, 0:256], a1[0:1, 0:256])
    a3 = scratch("a3")
    nc.vector.tensor_add(a3[:], a2[0:1, 0:128], a2[0:1, 128:256])
    a4 = scratch("a4")
    nc.vector.tensor_add(a4[:], a3[:], as2[0:1, 256:384])
    a5 = scratch("a5")
    nc.vector.tensor_scalar(a5[:], a4[:], LN2,
                            (BIAS_BITS * L - 8 * 127) * LN2, OP.mult, OP.add)
    t7 = scratch("t7")
    nc.vector.tensor_add(t7[:], t6[:], a5[:])
    ans = scratch("ans")
    nc.vector.tensor_sub(ans[:], t7[:], goldp[0:1, :])
    nc.sync.dma_start(out_ext.rearrange("(p x) -> p x", p=1), ans[:])


def _chunk_of(r):
    for i, (lo, hi) in enumerate(CHS):
        if lo <= r < hi:
            return i
    raise ValueError(r)


def build():
    if "nc" in _CACHE:
        return _CACHE["nc"]
    import concourse.bass as bass
    import concourse.tile as tile
    from concourse import bacc, mybir

    f32 = mybir.dt.float32
    bf16 = mybir.dt.bfloat16
    nc = bacc.Bacc("TRN2", debug=False)
    nc.all_engine_barrier()
    CBW = 128 + 128 + COLS + 2 + 1
    fd = nc.dram_tensor("fd", [128, SEG * COLS], bf16, kind="ExternalInput").ap()
    cbd = nc.dram_tensor("cb", [128, CBW], bf16, kind="ExternalInput").ap()
    seld = nc.dram_tensor("sel", [2, 130], f32, kind="ExternalInput").ap()
    goldd = nc.dram_tensor("gold", [128, 9 * 128], bf16,
                           kind="ExternalInput").ap()
    out_ext = nc.dram_tensor("out", [128], f32, kind="ExternalOutput").ap()
    dram = (fd, cbd, seld, goldd, out_ext)
    with ExitStack() as ctx:
        tc = ctx.enter_context(tile.TileContext(nc))
        _emit(ctx, tc, nc, mybir, bass, dram)
    nc.compile()
    _CACHE["nc"] = nc
    return nc


def host_prepare(feats, tags, transition):
    """Data rearrangement/gather only (plus tiny O(T^2) constant tables)."""
    feats = np.asarray(feats, dtype=np.float32)
    tags = np.asarray(tags)
    trans = np.asarray(transition, dtype=np.float32)

    # FD[c, p, r, col]: col = s*128 + j, lane = 128c + j
    #   p < 64:  feats[64s + r,        lane, p]     (fwd chain of seg s)
    #   p >= 64: feats[64(s+1)+63-r,   lane, p-64]  (bwd chain of seg s+1)
    ft = feats.transpose(2, 0, 1)                      # (T, L, B)
    ftr = ft.reshape(TAG, S, SEG, NCORE, 128)
    fwd = ftr[:, 0:NPAIR]
    bwd = ftr[:, 1:S, ::-1]
    FD = np.concatenate([fwd, bwd], axis=0)            # (128, s, r, c, j)
    FD = FD.transpose(3, 0, 2, 1, 4)                   # (c, p, r, s, j)
    FD = np.ascontiguousarray(FD).reshape(NCORE, 128, SEG * COLS).astype(BF16)

    E8 = (np.exp(trans) * 2.0 ** -BIAS_BITS).astype(np.float32)
    W = np.zeros((128, 128), np.float32)
    W[0:64, 0:64] = E8.T
    W[64:128, 64:128] = E8
    Wfin = np.zeros((128, 128), np.float32)
    Wfin[0:64, 64:128] = E8.T
    init = np.zeros((128, COLS), np.float32)
    rsum = E8.sum(axis=1)
    eend = np.exp(trans[END, :])
    for s in range(NPAIR):
        init[0:64, s * 128:(s + 1) * 128] = \
            (E8[:, START] if s == 0 else rsum)[:, None]
        init[64:128, s * 128:(s + 1) * 128] = \
            (eend if s == NPAIR - 1 else np.ones(TAG, np.float32))[:, None]
    onesbd = np.zeros((128, 2), np.float32)
    onesbd[0:64, 0] = 1.0
    onesbd[64:128, 1] = 1.0
    onesfull = np.ones((128, 1), np.float32)
    cb = np.concatenate([W, Wfin, init, onesbd, onesfull],
                        axis=1).astype(BF16)

    sel = np.zeros((2, 130), np.float32)
    sel[0, 0:64] = 1.0
    sel[1, 64:128] = 1.0
    sel[:, 128] = 1.0

    tags_ext = np.concatenate(
        [np.full((1, B), START, tags.dtype), tags], axis=0)
    emit = np.take_along_axis(
        feats, tags_ext[1:][:, :, None].astype(np.int64), axis=2)[..., 0]
    trg = trans[tags_ext[1:], tags_ext[:-1]]
    endt = trans[END, tags[-1]]
    gb = np.zeros((128, 9, NCORE, 128), np.float32)
    gb[:, 0:4] = emit.reshape(4, 128, NCORE, 128).transpose(1, 0, 2, 3)
    gb[:, 4:8] = trg.reshape(4, 128, NCORE, 128).transpose(1, 0, 2, 3)
    gb[0, 8] = endt.reshape(NCORE, 128)
    GOLD = np.ascontiguousarray(
        gb.transpose(2, 0, 1, 3)).reshape(NCORE, 128, 9 * 128).astype(BF16)
    return FD, cb, sel, GOLD


def _install_ntff_hook():
    """Provide antenv.axon_hooks (absent in this image) so trace=True can
    capture NTFF profiles via the axon .so C ABI."""
    import sys, types, ctypes, contextlib
    if "antenv.axon_hooks" in sys.modules:
        return
    so_path = None
    for line in open("/proc/self/maps"):
        if "libaxon_pjrt.so" in line:
            so_path = line.split()[-1]
            break
    mod = types.ModuleType("antenv.axon_hooks")
    state = {"hook": None}
    if so_path:
        lib = ctypes.CDLL(so_path)
        if hasattr(lib, "axon_start_nrt_profile"):
            lib.axon_start_nrt_profile.argtypes = [
                ctypes.POINTER(ctypes.c_int64), ctypes.c_size_t]
            lib.axon_start_nrt_profile.restype = ctypes.c_int64
            lib.axon_stop_nrt_profile.argtypes = [ctypes.c_char_p]
            lib.axon_stop_nrt_profile.restype = ctypes.c_int64

            @contextlib.contextmanager
            def _hook(output_dir, device_ids):
                import jax
                jax.devices()
                if device_ids:
                    ids = (ctypes.c_int64 * len(device_ids))(*device_ids)
                    rc = lib.axon_start_nrt_profile(ids, len(device_ids))
                else:
                    rc = lib.axon_start_nrt_profile(None, 0)
                if rc != 0:
                    raise RuntimeError(f"axon_start_nrt_profile rc={rc}")
                try:
                    yield
                finally:
                    n = lib.axon_stop_nrt_profile(str(output_dir).encode())
                    print(f"ntff profile: {n} file(s) -> {output_dir}")

            state["hook"] = _hook
    mod.get_axon_ntff_profile_hook = lambda: state["hook"]
    mod.set_axon_ntff_profile_hook = lambda h: state.update(hook=h)
    sys.modules["antenv.axon_hooks"] = mod


def kernel(feats, tags, mask, transition):
    from concourse.bass_utils import run_bass_kernel_spmd
    if os.environ.get("CRF_TRACE", "0") == "1":
        _install_ntff_hook()

    tags_np = np.asarray(tags)
    FD, cb, sel, GOLD = host_prepare(feats, tags_np, transition)
    nc = build()
    in_maps = []
    for c in range(NCORE):
        in_maps.append({"fd": FD[c], "cb": cb, "sel": sel, "gold": GOLD[c]})
    res = run_bass_kernel_spmd(nc, in_maps, list(range(NCORE)),
                               trace=bool(int(os.environ.get("CRF_TRACE", "0"))))
    out = np.concatenate([np.asarray(res.results[c]["out"]).reshape(128)
                          for c in range(NCORE)])
    if getattr(res, "exec_time_ns", None):
        print(f"HW exec time: {res.exec_time_ns} ns")
    return out.astype(np.float32)


# revision 27
# speedup vs baseline: 1.6757x; 1.0368x over previous
"""CRF NLL (allpath - realpath) Trainium2 Bass kernel, 8-core data parallel.

v2: segmented forward algorithm.  The sequential depth of the forward
recurrence is cut 4x by splitting the 512-step chain into S=8 segments of
64 steps.  Products of 64 positive transfer matrices are numerically rank-1
(Perron-Frobenius contraction; measured error ~1e-12 in fp64), so each
interior segment product P_s is replaced by (P_s g)(h^T P_s)/(h^T P_s g)
with probe vectors g = h = ones.  This yields 7 forward chains and 7
backward chains, all independent, each 64 steps deep:

  Z = e^T P_7 ... P_0 s0
    ~= (w^T a_6)(b_6^T a_5)...(b_1^T a_0) / (c_1 ... c_6)

where a_s = P_s g (a_0 uses the true s0), b_s^T = h^T P_s (b_7 uses the
true e = exp(trans[END])), and c_s = h^T a_s (the fwd-chain mass, free).

Chains run in scaled probability space: each step is one TensorEngine
matmul (stationary block-diag W = diag(E^T, E), E = exp(transition)*2^-8)
plus one DVE multiply with exp(feat).  Chains are packed in pairs
[fwd seg s ; bwd seg s+1] stacked on the 128 partitions, so every
inter-segment dot b_{s+1}^T a_s is a column-aligned top-half/bottom-half
product inside one tile.  The 7 pairs ride in two wide tiles (FD=512 and
FD=384); per round the engines see 2 matmuls + 2 DVE multiplies for all
14 chains, amortizing the DVE per-instruction fixed cost.

One power-of-2 renorm per chain at round 31 (mass via ones-matmul,
exponent extract via bit tricks on DVE/GPSIMD, scale folded lazily into a
future exp(feat) slice) keeps bf16 in range; exponents accumulate in int32
and fold into the final log.  Interior fwd-chain exponents cancel against
the c_s terms and are not tracked.

The gold-path score is a pure gather (no FP): the host ships
feats[l,b,tags[l,b]], transition[tag_{l+1},tag_l] and the END term as a
bf16 blob; the device reduces it with 9 accumulating ones-matmuls.
Host-side work is only data rearrangement/gather plus tiny O(T^2)
constant tables, as in v1; all O(L*B*T) floating-point math is on device.
"""
import os
import numpy as np
import ml_dtypes
from contextlib import ExitStack

BF16 = ml_dtypes.bfloat16

L, B, TAG = 512, 1024, 64
START, END = 62, 63
NCORE = 8
S = 8                   # segments
SEG = L // S            # 64 rounds
NPAIR = S - 1           # 7 chain pairs
FD1, FD2 = 512, 384     # tile1 = pairs 0-3, tile2 = pairs 4-6
COLS = NPAIR * 128      # 896
BIAS_BITS = 8.0
RENORM_AT = SEG // 2 - 1            # measure at round 31
APPLY_AT = RENORM_AT + 5            # fold scale into in1 of round 36
LN2 = float(np.log(2.0))
CHS = [(0, 2), (2, 4), (4, 8), (8, 12), (12, 16), (16, 24), (24, 32),
       (32, 40), (40, 48), (48, 56), (56, 64)]
NBUF_IN1 = 4

_CACHE = {}


def _emit(ctx, tc, nc, mybir, bass, dram):
    f32 = mybir.dt.float32
    i32 = mybir.dt.int32
    bf16 = mybir.dt.bfloat16
    AF = mybir.ActivationFunctionType
    OP = mybir.AluOpType

    fp8 = mybir.dt.float8e4
    fd, cbd, seld, goldd, out_ext = dram

    consts = ctx.enter_context(tc.tile_pool(name="consts", bufs=1))
    fd_pool = ctx.enter_context(tc.tile_pool(name="fd", bufs=2))
    in1_pool = ctx.enter_context(tc.tile_pool(name="in1", bufs=NBUF_IN1))
    st_pool = ctx.enter_context(tc.tile_pool(name="state", bufs=4))
    sm_pool = ctx.enter_context(tc.tile_pool(name="small", bufs=1))
    sc_pool = ctx.enter_context(tc.tile_pool(name="sync", bufs=2))
    q_pool = ctx.enter_context(tc.tile_pool(name="qpsum", bufs=2, space="PSUM"))
    ax_pool = ctx.enter_context(tc.tile_pool(name="axpsum", bufs=2, space="PSUM"))

    # --- sync absorbers (see v1): a 1-row dummy read makes the reading
    # engine's clock observe a producer's semaphore so Tile can elide that
    # wait from later ops on the same engine.
    def dve_sync(ap_slice):
        t = sc_pool.tile([1, 128], f32, tag="dsync")
        nc.vector.tensor_copy(t[:, 0:ap_slice.shape[-1]], ap_slice)

    def act_sync(ap_slice):
        t = sc_pool.tile([1, 128], f32, tag="async")
        nc.scalar.copy(t[:, 0:ap_slice.shape[-1]], ap_slice)

    def pool_sync(ap_slice):
        t = sc_pool.tile([1, 128], f32, tag="psync")
        nc.gpsimd.tensor_copy(t[:, 0:ap_slice.shape[-1]], ap_slice)

    # --- constants -------------------------------------------------------
    # cb layout: W | Wfin | init | onesbd | onesfull | ones2row | inv64
    CBW = 128 + 128 + COLS + 2 + 1 + 1 + 1
    cstage = sm_pool.tile([128, CBW], bf16, tag="cstage")
    cblob = consts.tile([128, CBW], bf16, tag="cblob")
    nc.sync.dma_start(cstage[:], cbd[:])
    nc.vector.tensor_copy(cblob[:], cstage[:])
    W_t = cblob[:, 0:128]
    Wfin_t = cblob[:, 128:256]
    init_t = cblob[:, 256:256 + COLS]
    onesbd_t = cblob[:, 256 + COLS:258 + COLS]
    ones64_t = cblob[64:128, 257 + COLS:258 + COLS]   # onesbd col 1, bottom half
    onesfull_t = cblob[:, 258 + COLS:259 + COLS]
    ones2_t = cblob[0:2, 259 + COLS:260 + COLS]       # [2,1] ones (rows 0-1)
    inv64_t = cblob[:, 260 + COLS:261 + COLS]         # 1/64 on rows 64-127

    acc_t = consts.tile([2, COLS], i32, tag="acc")
    sh23_t = consts.tile([2, COLS], i32, tag="sh23")
    nc.vector.memset(acc_t[:], 0)
    nc.vector.memset(sh23_t[:], 23)

    # --- per-chunk prep --------------------------------------------------
    preps = {}

    def prep(ci, sf_cur, pieces=None):
        lo, hi = CHS[ci]
        n = hi - lo
        fd_t = fd_pool.tile([128, n * COLS], fp8, tag="fd")
        nc.sync.dma_start(fd_t[:], fd[:, lo * COLS:hi * COLS])
        if sf_cur is not None:
            act_sync(sf_cur[0:1, 0:1])     # absorb DVE (in1 buffer WAR)
        act_sync(fd_t[0:1, 0:1])           # absorb fd DMA into ACT
        in1_t = in1_pool.tile([128, n * COLS], bf16, tag="in1")
        in1_3d = in1_t.rearrange("p (k x) -> p k x", x=COLS)
        fd_3d = fd_t.rearrange("p (k x) -> p k x", x=COLS)
        for k0, k1 in (pieces or [(0, n)]):
            nc.scalar.activation(in1_3d[:, k0:k1, :], fd_3d[:, k0:k1, :],
                                 AF.Exp)
        preps[ci] = in1_t
        return in1_t

    def prep_sync(in1_t):
        dve_sync(in1_t[0:1, 0:1])          # absorb ACT into DVE

    # --- startup (DMA order: cb, fd0, sel, fd1, fd2, gold) ---------------
    prep(0, None, pieces=[(0, 1), (1, 2)])
    selstage = sm_pool.tile([2, 128], f32, tag="selstage")
    sel_t = consts.tile([2, 128], f32, tag="sel")
    nc.sync.dma_start(selstage[:], seld[:])
    nc.vector.tensor_copy(sel_t[:], selstage[:])
    prep(1, None)
    prep(2, None)
    goldsb = consts.tile([128, 9 * 128], bf16, tag="goldsb")
    nc.sync.dma_start(goldsb[:], goldd[:])
    gold3d = goldsb.rearrange("p (g x) -> p g x", x=384)
    prep_sync(preps[0])

    in1c = preps[0]
    in1c_3d = in1c.rearrange("p (k x) -> p k x", x=COLS)
    sA = st_pool.tile([128, FD1], bf16, tag="stA")
    nc.vector.tensor_tensor(sA[:], init_t[:, 0:FD1], in1c_3d[:, 0, 0:FD1],
                            OP.mult)
    sB = st_pool.tile([128, FD2], bf16, tag="stB")
    nc.vector.tensor_tensor(sB[:], init_t[:, FD1:COLS], in1c_3d[:, 0, FD1:COLS],
                            OP.mult)

    emit_at = {CHS[1][0]: 3, CHS[2][0]: 4, CHS[3][0]: 5, CHS[4][0]: 6,
               CHS[5][0]: 7, CHS[6][0]: 8, CHS[7][0]: 9, CHS[8][0]: 10}
    ci = 0
    renorm_state = {}
    for r in range(1, SEG):
        if r in emit_at:
            prep(emit_at[r], sA)
        lo, hi = CHS[ci]
        if r >= hi:
            ci += 1
            lo, hi = CHS[ci]
            prep_sync(preps[ci])       # absorb this chunk's exp into DVE
            in1c = preps.pop(ci)
            in1c_3d = in1c.rearrange("p (k x) -> p k x", x=COLS)
        k = r - lo

        qA = q_pool.tile([128, FD1], f32, tag="q1")
        nc.tensor.matmul(qA[:], W_t, sA[:], start=True, stop=True)
        sA_new = st_pool.tile([128, FD1], bf16, tag="stA")
        nc.vector.tensor_mul(sA_new[:], qA[:], in1c_3d[:, k, 0:FD1])
        sA = sA_new

        qB = q_pool.tile([128, FD2], f32, tag="q2")
        nc.tensor.matmul(qB[:], W_t, sB[:], start=True, stop=True)
        sB_new = st_pool.tile([128, FD2], bf16, tag="stB")
        nc.vector.tensor_mul(sB_new[:], qB[:], in1c_3d[:, k, FD1:COLS])
        sB = sB_new

        if r == RENORM_AT:
            massA = ax_pool.tile([2, FD1], f32, tag="mass")
            nc.tensor.matmul(massA[:], onesbd_t, sA[:], start=True, stop=True)
            massB = ax_pool.tile([2, FD2], f32, tag="mass")
            nc.tensor.matmul(massB[:], onesbd_t, sB[:], start=True, stop=True)
            renorm_state["mass"] = (massA, massB)
        elif r == RENORM_AT + 2:
            massA, massB = renorm_state.pop("mass")
            eintA = sm_pool.tile([2, FD1], i32, tag="eintA")
            nc.vector.tensor_tensor(eintA[:], massA.bitcast(i32)[:],
                                    sh23_t[:, 0:FD1], OP.logical_shift_right)
            eintB = sm_pool.tile([2, FD2], i32, tag="eintB")
            nc.vector.tensor_tensor(eintB[:], massB.bitcast(i32)[:],
                                    sh23_t[:, 0:FD2], OP.logical_shift_right)
            pool_sync(eintB[0:1, 0:1])     # absorb DVE into GPSIMD
            sbitsA = sm_pool.tile([2, FD1], i32, tag="sbitsA")
            nc.gpsimd.tensor_scalar(sbitsA[:], eintA[:], -(1 << 23),
                                    0x7F000000, OP.mult, OP.add)
            sbitsB = sm_pool.tile([2, FD2], i32, tag="sbitsB")
            nc.gpsimd.tensor_scalar(sbitsB[:], eintB[:], -(1 << 23),
                                    0x7F000000, OP.mult, OP.add)
            # acc tracks pair-0 fwd (exact) + all bwd rows only; interior fwd
            # exponents cancel against the c_s mass terms.  Row-1-only slices
            # are illegal (partition base must be 32-aligned), so zero the
            # unwanted row-0 entries and accumulate full 2-row tiles.
            nc.vector.memset(eintA[0:1, 128:FD1], 0)
            nc.vector.memset(eintB[0:1, 0:FD2], 0)
            pool_sync(eintB[0:1, 0:1])     # absorb the memsets into GPSIMD
            nc.gpsimd.tensor_tensor(acc_t[:, 0:FD1], acc_t[:, 0:FD1],
                                    eintA[:], OP.add)
            nc.gpsimd.tensor_tensor(acc_t[:, FD1:COLS], acc_t[:, FD1:COLS],
                                    eintB[:], OP.add)
            renorm_state["sbits"] = (sbitsA, sbitsB)
        elif r == APPLY_AT - 2:
            sbitsA, sbitsB = renorm_state.pop("sbits")
            sbcA = q_pool.tile([128, FD1], f32, tag="q1")
            nc.tensor.matmul(sbcA[:], sel_t[:, 0:128], sbitsA.bitcast(f32)[:],
                             start=True, stop=True)
            sbcB = q_pool.tile([128, FD2], f32, tag="q2")
            nc.tensor.matmul(sbcB[:], sel_t[:, 0:128], sbitsB.bitcast(f32)[:],
                             start=True, stop=True)
            renorm_state["sbc"] = (sbcA, sbcB)
        elif r == APPLY_AT - 1:
            # fold 2^-e into the NEXT round's in1 slice; emitted here so it
            # precedes round APPLY_AT's chain TTs on the DVE queue
            sbcA, sbcB = renorm_state.pop("sbc")
            tci = _chunk_of(APPLY_AT)
            tgt = preps[tci] if tci in preps else in1c
            tgt3d = tgt.rearrange("p (k x) -> p k x", x=COLS)
            kk = APPLY_AT - CHS[tci][0]
            nc.vector.tensor_mul(tgt3d[:, kk, 0:FD1], tgt3d[:, kk, 0:FD1],
                                 sbcA[:])
            nc.vector.tensor_mul(tgt3d[:, kk, FD1:COLS], tgt3d[:, kk, FD1:COLS],
                                 sbcB[:])
        elif r == 44:
            # accf/accs run here (acc final since r==33); bf16 is exact for
            # these small biased-exponent sums
            accf = sm_pool.tile([2, COLS], bf16, tag="accf")
            nc.vector.tensor_copy(accf[:, 0:FD1], acc_t[:, 0:FD1])
            renorm_state["accf"] = accf
        elif r == 45:
            accf = renorm_state["accf"]
            nc.vector.tensor_copy(accf[:, FD1:COLS], acc_t[:, FD1:COLS])
        elif r == 46:
            accf = renorm_state.pop("accf")
            accs1 = ax_pool.tile([2, FD1], f32, tag="mass")
            nc.tensor.matmul(accs1[0:1, :], ones2_t, accf[:, 0:FD1],
                             start=True, stop=True)
            accs2 = ax_pool.tile([2, FD2], f32, tag="mass")
            nc.tensor.matmul(accs2[0:1, :], ones2_t, accf[:, FD1:COLS],
                             start=True, stop=True)
            renorm_state["accs"] = (accs1, accs2)

    # warm the Ln table while ACT is idle (ACT is in-order, so this lands
    # after the final Exp and the table switch is paid before the tail)
    lnwarm = sm_pool.tile([1, 2], f32, tag="lnwarm")
    nc.scalar.activation(lnwarm[:], in1c[0:1, 0:2], AF.Ln)

    # --- tail ------------------------------------------------------------
    # PE order: Wfin MMs, cmass MMs (+pair-0 1.0-block), gold MMs (fill the
    # pA/pB wait), meet MMs.  ACT: Ln(C-blob), Ln(meet-blob).  DVE: pA/pB,
    # then acc/gold folds during the Ln window, then the wide combine.
    accs1, accs2 = renorm_state.pop("accs")
    q2A = q_pool.tile([128, FD1], f32, tag="q1")
    nc.tensor.matmul(q2A[:], Wfin_t, sA[:], start=True, stop=True)
    q2B = q_pool.tile([128, FD2], f32, tag="q2")
    nc.tensor.matmul(q2B[:], Wfin_t, sB[:], start=True, stop=True)
    pA = st_pool.tile([128, FD1], bf16, tag="pA")
    nc.vector.tensor_mul(pA[64:128, :], q2A[64:128, :], sA[64:128, :])
    pB = st_pool.tile([128, FD2], bf16, tag="pB")
    nc.vector.tensor_mul(pB[64:128, :], q2B[64:128, :], sB[64:128, :])

    # C-mass blob: cols 0:128 = exact 1.0 (64 * 1/64), 128:512 = pairs 1-3,
    # 512:896 = pairs 4-6 -- block-aligned with the meet blob
    mc2 = ax_pool.tile([1, COLS], f32, tag="mass", name="mc2")
    nc.tensor.matmul(mc2[0:1, 0:128], inv64_t, init_t[:, 128:256],
                     start=True, stop=True)
    nc.tensor.matmul(mc2[0:1, 128:FD1], onesbd_t[:, 0:1], sA[:, 128:FD1],
                     start=True, stop=True)
    nc.tensor.matmul(mc2[0:1, FD1:COLS], onesbd_t[:, 0:1], sB[:],
                     start=True, stop=True)
    goldt = q_pool.tile([2, FD2], f32, tag="q2", name="goldt")
    goldp = goldt[0:1, 0:384]
    for g in range(3):
        nc.tensor.matmul(goldp, onesfull_t, gold3d[:, g, :],
                         start=(g == 0), stop=(g == 2))
    mc1 = ax_pool.tile([1, COLS], f32, tag="mass", name="mc1")
    nc.tensor.matmul(mc1[0:1, 0:FD1], ones64_t, pA[64:128, :], start=True,
                     stop=True)
    nc.tensor.matmul(mc1[0:1, FD1:COLS], ones64_t, pB[64:128, :], start=True,
                     stop=True)

    act_sync(mc2[0:1, 0:1])                # absorb PE into ACT
    lnC = sm_pool.tile([1, COLS], f32, tag="lnC")
    nc.scalar.activation(lnC[:], mc2[0:1, :], AF.Ln)
    act_sync(mc1[0:1, 0:1])
    lnm = sm_pool.tile([1, COLS], f32, tag="lnm")
    nc.scalar.activation(lnm[:], mc1[0:1, :], AF.Ln)

    # small f32 scratch tiles
    def scratch(tag, n=128):
        return sm_pool.tile([1, n], f32, tag=tag, name=tag)

    # acc + gold folds on DVE while ACT does the Lns
    dve_sync(goldt[0:1, 0:1])              # absorb PE into DVE
    as1 = scratch("as1", FD1)
    nc.vector.tensor_copy(as1[:], accs1[0:1, :])
    as2 = scratch("as2", FD2)
    nc.vector.tensor_copy(as2[:], accs2[0:1, :])
    gcopy = scratch("gcopy", FD2)
    nc.vector.tensor_copy(gcopy[:], goldp)
    a1 = scratch("a1", 256)
    nc.vector.tensor_add(a1[:], as1[0:1, 0:256], as1[0:1, 256:512])
    a2 = scratch("a2", 256)
    nc.vector.tensor_add(a2[:], as2[0:1, 0:256], a1[0:1, 0:256])
    a3 = scratch("a3")
    nc.vector.tensor_add(a3[:], a2[0:1, 0:128], a2[0:1, 128:256])
    a4 = scratch("a4")
    nc.vector.tensor_add(a4[:], a3[:], as2[0:1, 256:384])
    a5 = scratch("a5")
    nc.vector.tensor_scalar(a5[:], a4[:], LN2,
                            (BIAS_BITS * L - 8 * 127) * LN2, OP.mult, OP.add)
    g1 = scratch("g1")
    nc.vector.tensor_add(g1[:], gcopy[0:1, 0:128], gcopy[0:1, 128:256])
    g2 = scratch("g2")
    nc.vector.tensor_sub(g2[:], a5[:], g1[:])
    g3 = scratch("g3")
    nc.vector.tensor_sub(g3[:], g2[:], gcopy[0:1, 256:384])

    dve_sync(lnm[0:1, 0:1])                # absorb ACT into DVE
    d_all = sm_pool.tile([1, COLS], f32, tag="d_all")
    nc.vector.tensor_sub(d_all[:], lnm[:], lnC[:])
    u1 = scratch("u1", 384)
    nc.vector.tensor_add(u1[:], d_all[0:1, 0:384], d_all[0:1, 384:768])
    u2 = scratch("u2")
    nc.vector.tensor_add(u2[:], u1[0:1, 0:128], u1[0:1, 128:256])
    u3 = scratch("u3")
    nc.vector.tensor_add(u3[:], u2[:], u1[0:1, 256:384])
    u4 = scratch("u4")
    nc.vector.tensor_add(u4[:], u3[:], d_all[0:1, 768:896])
    ans = scratch("ans")
    nc.vector.tensor_add(ans[:], u4[:], g3[:])
    nc.sync.dma_start(out_ext.rearrange("(p x) -> p x", p=1), ans[:])


def _chunk_of(r):
    for i, (lo, hi) in enumerate(CHS):
        if lo <= r < hi:
            return i
    raise ValueError(r)


def build():
    if "nc" in _CACHE:
        return _CACHE["nc"]
    import concourse.bass as bass
    import concourse.tile as tile
    from concourse import bacc, mybir

    f32 = mybir.dt.float32
    bf16 = mybir.dt.bfloat16
    nc = bacc.Bacc("TRN2", debug=False)
    nc.all_engine_barrier()
    CBW = 128 + 128 + COLS + 2 + 1 + 1 + 1
    fd = nc.dram_tensor("fd", [128, SEG * COLS], mybir.dt.float8e4,
                        kind="ExternalInput").ap()
    cbd = nc.dram_tensor("cb", [128, CBW], bf16, kind="ExternalInput").ap()
    seld = nc.dram_tensor("sel", [2, 128], f32, kind="ExternalInput").ap()
    goldd = nc.dram_tensor("gold", [128, 9 * 128], bf16,
                           kind="ExternalInput").ap()
    out_ext = nc.dram_tensor("out", [128], f32, kind="ExternalOutput").ap()
    dram = (fd, cbd, seld, goldd, out_ext)
    with ExitStack() as ctx:
        tc = ctx.enter_context(tile.TileContext(nc))
        _emit(ctx, tc, nc, mybir, bass, dram)
    nc.compile()
    _CACHE["nc"] = nc
    return nc


def host_prepare(feats, tags, transition):
    """Data rearrangement/gather only (plus tiny O(T^2) constant tables)."""
    feats = np.asarray(feats, dtype=np.float32)
    tags = np.asarray(tags)
    trans = np.asarray(transition, dtype=np.float32)

    # FD[c, p, r, col]: col = s*128 + j, lane = 128c + j
    #   p < 64:  feats[64s + r,        lane, p]     (fwd chain of seg s)
    #   p >= 64: feats[64(s+1)+63-r,   lane, p-64]  (bwd chain of seg s+1)
    ft = feats.transpose(2, 0, 1)                      # (T, L, B)
    ftr = ft.reshape(TAG, S, SEG, NCORE, 128)
    fwd = ftr[:, 0:NPAIR]
    bwd = ftr[:, 1:S, ::-1]
    FD = np.concatenate([fwd, bwd], axis=0)            # (128, s, r, c, j)
    FD = FD.transpose(3, 0, 2, 1, 4)                   # (c, p, r, s, j)
    FD = np.ascontiguousarray(FD).reshape(
        NCORE, 128, SEG * COLS).astype(ml_dtypes.float8_e4m3)

    E8 = (np.exp(trans) * 2.0 ** -BIAS_BITS).astype(np.float32)
    W = np.zeros((128, 128), np.float32)
    W[0:64, 0:64] = E8.T
    W[64:128, 64:128] = E8
    Wfin = np.zeros((128, 128), np.float32)
    Wfin[0:64, 64:128] = E8.T
    init = np.zeros((128, COLS), np.float32)
    rsum = E8.sum(axis=1)
    eend = np.exp(trans[END, :])
    for s in range(NPAIR):
        init[0:64, s * 128:(s + 1) * 128] = \
            (E8[:, START] if s == 0 else rsum)[:, None]
        init[64:128, s * 128:(s + 1) * 128] = \
            (eend if s == NPAIR - 1 else np.ones(TAG, np.float32))[:, None]
    onesbd = np.zeros((128, 2), np.float32)
    onesbd[0:64, 0] = 1.0
    onesbd[64:128, 1] = 1.0
    onesfull = np.ones((128, 1), np.float32)
    ones2 = np.ones((128, 1), np.float32)
    inv64 = np.zeros((128, 1), np.float32)
    inv64[64:128, 0] = 1.0 / 64.0
    cb = np.concatenate([W, Wfin, init, onesbd, onesfull, ones2, inv64],
                        axis=1).astype(BF16)

    sel = np.zeros((2, 128), np.float32)
    sel[0, 0:64] = 1.0
    sel[1, 64:128] = 1.0

    tags_ext = np.concatenate(
        [np.full((1, B), START, tags.dtype), tags], axis=0)
    emit = np.take_along_axis(
        feats, tags_ext[1:][:, :, None].astype(np.int64), axis=2)[..., 0]
    trg = trans[tags_ext[1:], tags_ext[:-1]]
    endt = trans[END, tags[-1]]
    gb = np.zeros((128, 9, NCORE, 128), np.float32)
    gb[:, 0:4] = emit.reshape(4, 128, NCORE, 128).transpose(1, 0, 2, 3)
    gb[:, 4:8] = trg.reshape(4, 128, NCORE, 128).transpose(1, 0, 2, 3)
    gb[0, 8] = endt.reshape(NCORE, 128)
    GOLD = np.ascontiguousarray(
        gb.transpose(2, 0, 1, 3)).reshape(NCORE, 128, 9 * 128).astype(BF16)
    return FD, cb, sel, GOLD


def _install_ntff_hook():
    """Provide antenv.axon_hooks (absent in this image) so trace=True can
    capture NTFF profiles via the axon .so C ABI."""
    import sys, types, ctypes, contextlib
    if "antenv.axon_hooks" in sys.modules:
        return
    so_path = None
    for line in open("/proc/self/maps"):
        if "libaxon_pjrt.so" in line:
            so_path = line.split()[-1]
            break
    mod = types.ModuleType("antenv.axon_hooks")
    state = {"hook": None}
    if so_path:
        lib = ctypes.CDLL(so_path)
        if hasattr(lib, "axon_start_nrt_profile"):
            lib.axon_start_nrt_profile.argtypes = [
                ctypes.POINTER(ctypes.c_int64), ctypes.c_size_t]
            lib.axon_start_nrt_profile.restype = ctypes.c_int64
            lib.axon_stop_nrt_profile.argtypes = [ctypes.c_char_p]
            lib.axon_stop_nrt_profile.restype = ctypes.c_int64

            @contextlib.contextmanager
            def _hook(output_dir, device_ids):
                import jax
                jax.devices()
                if device_ids:
                    ids = (ctypes.c_int64 * len(device_ids))(*device_ids)
                    rc = lib.axon_start_nrt_profile(ids, len(device_ids))
                else:
                    rc = lib.axon_start_nrt_profile(None, 0)
                if rc != 0:
                    raise RuntimeError(f"axon_start_nrt_profile rc={rc}")
                try:
                    yield
                finally:
                    n = lib.axon_stop_nrt_profile(str(output_dir).encode())
                    print(f"ntff profile: {n} file(s) -> {output_dir}")

            state["hook"] = _hook
    mod.get_axon_ntff_profile_hook = lambda: state["hook"]
    mod.set_axon_ntff_profile_hook = lambda h: state.update(hook=h)
    sys.modules["antenv.axon_hooks"] = mod


def kernel(feats, tags, mask, transition):
    from concourse.bass_utils import run_bass_kernel_spmd
    if os.environ.get("CRF_TRACE", "0") == "1":
        _install_ntff_hook()

    tags_np = np.asarray(tags)
    FD, cb, sel, GOLD = host_prepare(feats, tags_np, transition)
    nc = build()
    in_maps = []
    for c in range(NCORE):
        in_maps.append({"fd": FD[c], "cb": cb, "sel": sel, "gold": GOLD[c]})
    res = run_bass_kernel_spmd(nc, in_maps, list(range(NCORE)),
                               trace=bool(int(os.environ.get("CRF_TRACE", "0"))))
    out = np.concatenate([np.asarray(res.results[c]["out"]).reshape(128)
                          for c in range(NCORE)])
    if getattr(res, "exec_time_ns", None):
        print(f"HW exec time: {res.exec_time_ns} ns")
    return out.astype(np.float32)


# revision 34
# speedup vs baseline: 1.7857x; 1.0656x over previous
"""CRF NLL (allpath - realpath) Trainium2 Bass kernel, 8-core data parallel.

v2: segmented forward algorithm.  The sequential depth of the forward
recurrence is cut 4x by splitting the 512-step chain into S=8 segments of
64 steps.  Products of 64 positive transfer matrices are numerically rank-1
(Perron-Frobenius contraction; measured error ~1e-12 in fp64), so each
interior segment product P_s is replaced by (P_s g)(h^T P_s)/(h^T P_s g)
with probe vectors g = h = ones.  This yields 7 forward chains and 7
backward chains, all independent, each 64 steps deep:

  Z = e^T P_7 ... P_0 s0
    ~= (w^T a_6)(b_6^T a_5)...(b_1^T a_0) / (c_1 ... c_6)

where a_s = P_s g (a_0 uses the true s0), b_s^T = h^T P_s (b_7 uses the
true e = exp(trans[END])), and c_s = h^T a_s (the fwd-chain mass, free).

Chains run in scaled probability space: each step is one TensorEngine
matmul (stationary block-diag W = diag(E^T, E), E = exp(transition)*2^-8)
plus one DVE multiply with exp(feat).  Chains are packed in pairs
[fwd seg s ; bwd seg s+1] stacked on the 128 partitions, so every
inter-segment dot b_{s+1}^T a_s is a column-aligned top-half/bottom-half
product inside one tile.  The 7 pairs ride in two wide tiles (FD=512 and
FD=384); per round the engines see 2 matmuls + 2 DVE multiplies for all
14 chains, amortizing the DVE per-instruction fixed cost.

One power-of-2 renorm per chain at round 31 (mass via ones-matmul,
exponent extract via bit tricks on DVE/GPSIMD, scale folded lazily into a
future exp(feat) slice) keeps bf16 in range; exponents accumulate in int32
and fold into the final log.  Interior fwd-chain exponents cancel against
the c_s terms and are not tracked.

The gold-path score is a pure gather (no FP): the host ships
feats[l,b,tags[l,b]], transition[tag_{l+1},tag_l] and the END term as a
bf16 blob; the device reduces it with 9 accumulating ones-matmuls.
Host-side work is only data rearrangement/gather plus tiny O(T^2)
constant tables, as in v1; all O(L*B*T) floating-point math is on device.
"""
import os
import numpy as np
import ml_dtypes
from contextlib import ExitStack

BF16 = ml_dtypes.bfloat16

L, B, TAG = 512, 1024, 64
START, END = 62, 63
NCORE = 8
S = 8                   # segments
SEG = L // S            # 64 rounds
NPAIR = S - 1           # 7 chain pairs
# chains are column-independent, so the 7 pairs split evenly: tile1 =
# cols 0:448 (pairs 0-2 + half of 3), tile2 = cols 448:896 -- balanced
# MM/TT sizes minimize the latency-bound round
FD1, FD2 = 448, 448
COLS = NPAIR * 128      # 896
BIAS_BITS = 8.0
RENORM_AT = SEG // 2 - 1            # measure at round 31
APPLY_AT = RENORM_AT + 5            # fold scale into in1 of round 36
LN2 = float(np.log(2.0))
CHS = [(0, 2), (2, 4), (4, 8), (8, 12), (12, 16), (16, 24), (24, 32),
       (32, 40), (40, 48), (48, 56), (56, 64)]
NBUF_IN1 = 4

_CACHE = {}


def _emit(ctx, tc, nc, mybir, bass, dram):
    f32 = mybir.dt.float32
    i32 = mybir.dt.int32
    bf16 = mybir.dt.bfloat16
    AF = mybir.ActivationFunctionType
    OP = mybir.AluOpType

    fp8 = mybir.dt.float8e4
    fd, cbd, seld, goldd, out_ext = dram

    consts = ctx.enter_context(tc.tile_pool(name="consts", bufs=1))
    fd_pool = ctx.enter_context(tc.tile_pool(name="fd", bufs=2))
    in1_pool = ctx.enter_context(tc.tile_pool(name="in1", bufs=NBUF_IN1))
    st_pool = ctx.enter_context(tc.tile_pool(name="state", bufs=4))
    sm_pool = ctx.enter_context(tc.tile_pool(name="small", bufs=1))
    sc_pool = ctx.enter_context(tc.tile_pool(name="sync", bufs=2))
    q_pool = ctx.enter_context(tc.tile_pool(name="qpsum", bufs=2, space="PSUM"))
    ax_pool = ctx.enter_context(tc.tile_pool(name="axpsum", bufs=2, space="PSUM"))

    # --- sync absorbers (see v1): a 1-row dummy read makes the reading
    # engine's clock observe a producer's semaphore so Tile can elide that
    # wait from later ops on the same engine.
    def dve_sync(ap_slice):
        t = sc_pool.tile([1, 128], f32, tag="dsync")
        nc.vector.tensor_copy(t[:, 0:ap_slice.shape[-1]], ap_slice)

    def act_sync(ap_slice):
        t = sc_pool.tile([1, 128], f32, tag="async")
        nc.scalar.copy(t[:, 0:ap_slice.shape[-1]], ap_slice)

    def pool_sync(ap_slice):
        t = sc_pool.tile([1, 128], f32, tag="psync")
        nc.gpsimd.tensor_copy(t[:, 0:ap_slice.shape[-1]], ap_slice)

    # --- constants -------------------------------------------------------
    # cb layout: W | Wfin | init | onesbd | onesfull | ones2row | inv64
    CBW = 128 + 128 + COLS + 2 + 1 + 1 + 1
    cstage = sm_pool.tile([128, CBW], bf16, tag="cstage")
    cblob = consts.tile([128, CBW], bf16, tag="cblob")

    def load_consts():
        nc.sync.dma_start(cstage[:], cbd[:])
        nc.vector.tensor_copy(cblob[:], cstage[:])
    W_t = cblob[:, 0:128]
    Wfin_t = cblob[:, 128:256]
    init_t = cblob[:, 256:256 + COLS]
    onesbd_t = cblob[:, 256 + COLS:258 + COLS]
    ones64_t = cblob[64:128, 257 + COLS:258 + COLS]   # onesbd col 1, bottom half
    onesfull_t = cblob[:, 258 + COLS:259 + COLS]
    ones2_t = cblob[0:2, 259 + COLS:260 + COLS]       # [2,1] ones (rows 0-1)
    inv64_t = cblob[:, 260 + COLS:261 + COLS]         # 1/64 on rows 64-127

    acc_t = consts.tile([2, COLS], i32, tag="acc")
    sh23_t = consts.tile([2, COLS], i32, tag="sh23")
    nc.vector.memset(acc_t[:], 0)
    nc.vector.memset(sh23_t[:], 23)

    # --- per-chunk prep --------------------------------------------------
    preps = {}

    def prep(ci, sf_cur, pieces=None):
        lo, hi = CHS[ci]
        n = hi - lo
        fd_t = fd_pool.tile([128, n * COLS], fp8, tag="fd")
        nc.sync.dma_start(fd_t[:], fd[:, lo * COLS:hi * COLS])
        if sf_cur is not None:
            act_sync(sf_cur[0:1, 0:1])     # absorb DVE (in1 buffer WAR)
        act_sync(fd_t[0:1, 0:1])           # absorb fd DMA into ACT
        in1_t = in1_pool.tile([128, n * COLS], bf16, tag="in1")
        in1_3d = in1_t.rearrange("p (k x) -> p k x", x=COLS)
        fd_3d = fd_t.rearrange("p (k x) -> p k x", x=COLS)
        for k0, k1 in (pieces or [(0, n)]):
            nc.scalar.activation(in1_3d[:, k0:k1, :], fd_3d[:, k0:k1, :],
                                 AF.Exp)
        preps[ci] = in1_t
        return in1_t

    def prep_sync(in1_t):
        dve_sync(in1_t[0:1, 0:1])          # absorb ACT into DVE

    # --- startup (DMA order: fd0, cb, sel, fd1, fd2, gold) ---------------
    prep(0, None, pieces=[(0, 1), (1, 2)])
    load_consts()
    selstage = sm_pool.tile([2, 128], f32, tag="selstage")
    sel_t = consts.tile([2, 128], f32, tag="sel")
    nc.sync.dma_start(selstage[:], seld[:])
    nc.vector.tensor_copy(sel_t[:], selstage[:])
    prep(1, None)
    prep(2, None)
    goldsb = consts.tile([128, 9 * 128], bf16, tag="goldsb")
    nc.sync.dma_start(goldsb[:], goldd[:])
    gold3d = goldsb.rearrange("p (g x) -> p g x", x=384)
    prep_sync(preps[0])

    in1c = preps[0]
    in1c_3d = in1c.rearrange("p (k x) -> p k x", x=COLS)
    sA = st_pool.tile([128, FD1], bf16, tag="stA")
    nc.vector.tensor_tensor(sA[:], init_t[:, 0:FD1], in1c_3d[:, 0, 0:FD1],
                            OP.mult)
    sB = st_pool.tile([128, FD2], bf16, tag="stB")
    nc.vector.tensor_tensor(sB[:], init_t[:, FD1:COLS], in1c_3d[:, 0, FD1:COLS],
                            OP.mult)

    emit_at = {CHS[1][0]: 3, CHS[2][0]: 4, CHS[3][0]: 5, CHS[4][0]: 6,
               CHS[5][0]: 7, CHS[6][0]: 8, CHS[7][0]: 9, CHS[8][0]: 10}
    ci = 0
    renorm_state = {}
    for r in range(1, SEG):
        if r in emit_at:
            prep(emit_at[r], sA)
        lo, hi = CHS[ci]
        if r >= hi:
            ci += 1
            lo, hi = CHS[ci]
            prep_sync(preps[ci])       # absorb this chunk's exp into DVE
            in1c = preps.pop(ci)
            in1c_3d = in1c.rearrange("p (k x) -> p k x", x=COLS)
        k = r - lo

        qA = q_pool.tile([128, FD1], f32, tag="q1")
        nc.tensor.matmul(qA[:], W_t, sA[:], start=True, stop=True)
        sA_new = st_pool.tile([128, FD1], bf16, tag="stA")
        nc.vector.tensor_mul(sA_new[:], qA[:], in1c_3d[:, k, 0:FD1])
        sA = sA_new

        qB = q_pool.tile([128, FD2], f32, tag="q2")
        nc.tensor.matmul(qB[:], W_t, sB[:], start=True, stop=True)
        sB_new = st_pool.tile([128, FD2], bf16, tag="stB")
        nc.vector.tensor_mul(sB_new[:], qB[:], in1c_3d[:, k, FD1:COLS])
        sB = sB_new

        if r == RENORM_AT:
            massA = ax_pool.tile([2, FD1], f32, tag="mass")
            nc.tensor.matmul(massA[:], onesbd_t, sA[:], start=True, stop=True)
            massB = ax_pool.tile([2, FD2], f32, tag="mass")
            nc.tensor.matmul(massB[:], onesbd_t, sB[:], start=True, stop=True)
            renorm_state["mass"] = (massA, massB)
        elif r == RENORM_AT + 2:
            massA, massB = renorm_state.pop("mass")
            eintA = sm_pool.tile([2, FD1], i32, tag="eintA")
            nc.vector.tensor_tensor(eintA[:], massA.bitcast(i32)[:],
                                    sh23_t[:, 0:FD1], OP.logical_shift_right)
            eintB = sm_pool.tile([2, FD2], i32, tag="eintB")
            nc.vector.tensor_tensor(eintB[:], massB.bitcast(i32)[:],
                                    sh23_t[:, 0:FD2], OP.logical_shift_right)
            pool_sync(eintB[0:1, 0:1])     # absorb DVE into GPSIMD
            sbitsA = sm_pool.tile([2, FD1], i32, tag="sbitsA")
            nc.gpsimd.tensor_scalar(sbitsA[:], eintA[:], -(1 << 23),
                                    0x7F000000, OP.mult, OP.add)
            sbitsB = sm_pool.tile([2, FD2], i32, tag="sbitsB")
            nc.gpsimd.tensor_scalar(sbitsB[:], eintB[:], -(1 << 23),
                                    0x7F000000, OP.mult, OP.add)
            # acc tracks pair-0 fwd (exact) + all bwd rows only; interior fwd
            # exponents cancel against the c_s mass terms.  Row-1-only slices
            # are illegal (partition base must be 32-aligned), so zero the
            # unwanted row-0 entries and accumulate full 2-row tiles.
            nc.vector.memset(eintA[0:1, 128:FD1], 0)
            nc.vector.memset(eintB[0:1, 0:FD2], 0)
            pool_sync(eintB[0:1, 0:1])     # absorb the memsets into GPSIMD
            nc.gpsimd.tensor_tensor(acc_t[:, 0:FD1], acc_t[:, 0:FD1],
                                    eintA[:], OP.add)
            nc.gpsimd.tensor_tensor(acc_t[:, FD1:COLS], acc_t[:, FD1:COLS],
                                    eintB[:], OP.add)
            renorm_state["sbits"] = (sbitsA, sbitsB)
        elif r == APPLY_AT - 2:
            sbitsA, sbitsB = renorm_state.pop("sbits")
            sbcA = q_pool.tile([128, FD1], f32, tag="q1")
            nc.tensor.matmul(sbcA[:], sel_t[:, 0:128], sbitsA.bitcast(f32)[:],
                             start=True, stop=True)
            sbcB = q_pool.tile([128, FD2], f32, tag="q2")
            nc.tensor.matmul(sbcB[:], sel_t[:, 0:128], sbitsB.bitcast(f32)[:],
                             start=True, stop=True)
            renorm_state["sbc"] = (sbcA, sbcB)
        elif r == APPLY_AT - 1:
            # fold 2^-e into the NEXT round's in1 slice; emitted here so it
            # precedes round APPLY_AT's chain TTs on the DVE queue
            sbcA, sbcB = renorm_state.pop("sbc")
            tci = _chunk_of(APPLY_AT)
            tgt = preps[tci] if tci in preps else in1c
            tgt3d = tgt.rearrange("p (k x) -> p k x", x=COLS)
            kk = APPLY_AT - CHS[tci][0]
            nc.vector.tensor_mul(tgt3d[:, kk, 0:FD1], tgt3d[:, kk, 0:FD1],
                                 sbcA[:])
            nc.vector.tensor_mul(tgt3d[:, kk, FD1:COLS], tgt3d[:, kk, FD1:COLS],
                                 sbcB[:])
        elif r == 44:
            # accf/accs run here (acc final since r==33); bf16 is exact for
            # these small biased-exponent sums
            accf = sm_pool.tile([2, COLS], bf16, tag="accf")
            nc.vector.tensor_copy(accf[:, 0:FD1], acc_t[:, 0:FD1])
            renorm_state["accf"] = accf
        elif r == 45:
            accf = renorm_state["accf"]
            nc.vector.tensor_copy(accf[:, FD1:COLS], acc_t[:, FD1:COLS])
        elif r == 46:
            accf = renorm_state.pop("accf")
            accs1 = ax_pool.tile([2, FD1], f32, tag="mass")
            nc.tensor.matmul(accs1[0:1, :], ones2_t, accf[:, 0:FD1],
                             start=True, stop=True)
            accs2 = ax_pool.tile([2, FD2], f32, tag="mass")
            nc.tensor.matmul(accs2[0:1, :], ones2_t, accf[:, FD1:COLS],
                             start=True, stop=True)
            renorm_state["accs"] = (accs1, accs2)

    # warm the Ln table while ACT is idle (ACT is in-order, so this lands
    # after the final Exp and the table switch is paid before the tail)
    lnwarm = sm_pool.tile([1, 2], f32, tag="lnwarm")
    nc.scalar.activation(lnwarm[:], in1c[0:1, 0:2], AF.Ln)

    # --- tail ------------------------------------------------------------
    # PE order: Wfin MMs, cmass MMs (+pair-0 1.0-block), gold MMs (fill the
    # pA/pB wait), meet MMs.  ACT: Ln(C-blob), Ln(meet-blob).  DVE: pA/pB,
    # then acc/gold folds during the Ln window, then the wide combine.
    accs1, accs2 = renorm_state.pop("accs")
    q2A = q_pool.tile([128, FD1], f32, tag="q1")
    nc.tensor.matmul(q2A[:], Wfin_t, sA[:], start=True, stop=True)
    q2B = q_pool.tile([128, FD2], f32, tag="q2")
    nc.tensor.matmul(q2B[:], Wfin_t, sB[:], start=True, stop=True)
    pA = st_pool.tile([128, FD1], bf16, tag="pA")
    nc.vector.tensor_mul(pA[64:128, :], q2A[64:128, :], sA[64:128, :])
    pB = st_pool.tile([128, FD2], bf16, tag="pB")
    nc.vector.tensor_mul(pB[64:128, :], q2B[64:128, :], sB[64:128, :])

    # C-mass blob: cols 0:128 = exact 1.0 (64 * 1/64), 128:512 = pairs 1-3,
    # 512:896 = pairs 4-6 -- block-aligned with the meet blob
    # (PSUM matmul outputs must stay within one 2KB bank: split at col 512)
    mc2 = ax_pool.tile([1, COLS], f32, tag="mass", name="mc2")
    nc.tensor.matmul(mc2[0:1, 0:128], inv64_t, init_t[:, 128:256],
                     start=True, stop=True)
    nc.tensor.matmul(mc2[0:1, 128:FD1], onesbd_t[:, 0:1], sA[:, 128:FD1],
                     start=True, stop=True)
    nc.tensor.matmul(mc2[0:1, FD1:512], onesbd_t[:, 0:1], sB[:, 0:512 - FD1],
                     start=True, stop=True)
    nc.tensor.matmul(mc2[0:1, 512:COLS], onesbd_t[:, 0:1],
                     sB[:, 512 - FD1:FD2], start=True, stop=True)
    goldt = q_pool.tile([2, FD2], f32, tag="q2", name="goldt")
    goldp = goldt[0:1, 0:384]
    for g in range(3):
        nc.tensor.matmul(goldp, onesfull_t, gold3d[:, g, :],
                         start=(g == 0), stop=(g == 2))
    mc1 = ax_pool.tile([1, COLS], f32, tag="mass", name="mc1")
    nc.tensor.matmul(mc1[0:1, 0:FD1], ones64_t, pA[64:128, :], start=True,
                     stop=True)
    nc.tensor.matmul(mc1[0:1, FD1:512], ones64_t, pB[64:128, 0:512 - FD1],
                     start=True, stop=True)
    nc.tensor.matmul(mc1[0:1, 512:COLS], ones64_t, pB[64:128, 512 - FD1:FD2],
                     start=True, stop=True)

    act_sync(mc2[0:1, 0:1])                # absorb PE into ACT
    lnC = sm_pool.tile([1, COLS], f32, tag="lnC")
    nc.scalar.activation(lnC[:], mc2[0:1, :], AF.Ln)
    act_sync(mc1[0:1, 0:1])
    lnm = sm_pool.tile([1, COLS], f32, tag="lnm")
    nc.scalar.activation(lnm[:], mc1[0:1, :], AF.Ln)

    # small f32 scratch tiles
    def scratch(tag, n=128):
        return sm_pool.tile([1, n], f32, tag=tag, name=tag)

    # acc + gold folds on DVE while ACT does the Lns
    dve_sync(goldt[0:1, 0:1])              # absorb PE into DVE
    as_all = scratch("as_all", COLS)
    nc.vector.tensor_copy(as_all[0:1, 0:FD1], accs1[0:1, :])
    nc.vector.tensor_copy(as_all[0:1, FD1:COLS], accs2[0:1, :])
    gcopy = scratch("gcopy", 384)
    nc.vector.tensor_copy(gcopy[:], goldp)
    a1 = scratch("a1", 384)
    nc.vector.tensor_add(a1[:], as_all[0:1, 0:384], as_all[0:1, 384:768])
    a2 = scratch("a2")
    nc.vector.tensor_add(a2[:], a1[0:1, 0:128], a1[0:1, 128:256])
    a3 = scratch("a3")
    nc.vector.tensor_add(a3[:], a2[:], a1[0:1, 256:384])
    a4 = scratch("a4")
    nc.vector.tensor_add(a4[:], a3[:], as_all[0:1, 768:896])
    a5 = scratch("a5")
    nc.vector.tensor_scalar(a5[:], a4[:], LN2,
                            (BIAS_BITS * L - 8 * 127) * LN2, OP.mult, OP.add)
    g1 = scratch("g1")
    nc.vector.tensor_add(g1[:], gcopy[0:1, 0:128], gcopy[0:1, 128:256])
    g2 = scratch("g2")
    nc.vector.tensor_sub(g2[:], a5[:], g1[:])
    g3 = scratch("g3")
    nc.vector.tensor_sub(g3[:], g2[:], gcopy[0:1, 256:384])

    dve_sync(lnm[0:1, 0:1])                # absorb ACT into DVE
    d_all = sm_pool.tile([1, COLS], f32, tag="d_all")
    nc.vector.tensor_sub(d_all[:], lnm[:], lnC[:])
    u1 = scratch("u1", 384)
    nc.vector.tensor_add(u1[:], d_all[0:1, 0:384], d_all[0:1, 384:768])
    u2 = scratch("u2")
    nc.vector.tensor_add(u2[:], u1[0:1, 0:128], u1[0:1, 128:256])
    u3 = scratch("u3")
    nc.vector.tensor_add(u3[:], u2[:], u1[0:1, 256:384])
    u4 = scratch("u4")
    nc.vector.tensor_add(u4[:], u3[:], d_all[0:1, 768:896])
    ans = scratch("ans")
    nc.vector.tensor_add(ans[:], u4[:], g3[:])
    nc.sync.dma_start(out_ext.rearrange("(p x) -> p x", p=1), ans[:])


def _chunk_of(r):
    for i, (lo, hi) in enumerate(CHS):
        if lo <= r < hi:
            return i
    raise ValueError(r)


def build():
    if "nc" in _CACHE:
        return _CACHE["nc"]
    import concourse.bass as bass
    import concourse.tile as tile
    from concourse import bacc, mybir

    f32 = mybir.dt.float32
    bf16 = mybir.dt.bfloat16
    nc = bacc.Bacc("TRN2", debug=False)
    nc.all_engine_barrier()
    CBW = 128 + 128 + COLS + 2 + 1 + 1 + 1
    fd = nc.dram_tensor("fd", [128, SEG * COLS], mybir.dt.float8e4,
                        kind="ExternalInput").ap()
    cbd = nc.dram_tensor("cb", [128, CBW], bf16, kind="ExternalInput").ap()
    seld = nc.dram_tensor("sel", [2, 128], f32, kind="ExternalInput").ap()
    goldd = nc.dram_tensor("gold", [128, 9 * 128], bf16,
                           kind="ExternalInput").ap()
    out_ext = nc.dram_tensor("out", [128], f32, kind="ExternalOutput").ap()
    dram = (fd, cbd, seld, goldd, out_ext)
    with ExitStack() as ctx:
        tc = ctx.enter_context(tile.TileContext(nc))
        _emit(ctx, tc, nc, mybir, bass, dram)
    nc.compile()
    _CACHE["nc"] = nc
    return nc


def host_prepare(feats, tags, transition):
    """Data rearrangement/gather only (plus tiny O(T^2) constant tables)."""
    feats = np.asarray(feats, dtype=np.float32)
    tags = np.asarray(tags)
    trans = np.asarray(transition, dtype=np.float32)

    # FD[c, p, r, col]: col = s*128 + j, lane = 128c + j
    #   p < 64:  feats[64s + r,        lane, p]     (fwd chain of seg s)
    #   p >= 64: feats[64(s+1)+63-r,   lane, p-64]  (bwd chain of seg s+1)
    ft = feats.transpose(2, 0, 1)                      # (T, L, B)
    ftr = ft.reshape(TAG, S, SEG, NCORE, 128)
    fwd = ftr[:, 0:NPAIR]
    bwd = ftr[:, 1:S, ::-1]
    FD = np.concatenate([fwd, bwd], axis=0)            # (128, s, r, c, j)
    FD = FD.transpose(3, 0, 2, 1, 4)                   # (c, p, r, s, j)
    FD = np.ascontiguousarray(FD).reshape(
        NCORE, 128, SEG * COLS).astype(ml_dtypes.float8_e4m3)

    E8 = (np.exp(trans) * 2.0 ** -BIAS_BITS).astype(np.float32)
    W = np.zeros((128, 128), np.float32)
    W[0:64, 0:64] = E8.T
    W[64:128, 64:128] = E8
    Wfin = np.zeros((128, 128), np.float32)
    Wfin[0:64, 64:128] = E8.T
    init = np.zeros((128, COLS), np.float32)
    rsum = E8.sum(axis=1)
    eend = np.exp(trans[END, :])
    for s in range(NPAIR):
        init[0:64, s * 128:(s + 1) * 128] = \
            (E8[:, START] if s == 0 else rsum)[:, None]
        init[64:128, s * 128:(s + 1) * 128] = \
            (eend if s == NPAIR - 1 else np.ones(TAG, np.float32))[:, None]
    onesbd = np.zeros((128, 2), np.float32)
    onesbd[0:64, 0] = 1.0
    onesbd[64:128, 1] = 1.0
    onesfull = np.ones((128, 1), np.float32)
    ones2 = np.ones((128, 1), np.float32)
    inv64 = np.zeros((128, 1), np.float32)
    inv64[64:128, 0] = 1.0 / 64.0
    cb = np.concatenate([W, Wfin, init, onesbd, onesfull, ones2, inv64],
                        axis=1).astype(BF16)

    sel = np.zeros((2, 128), np.float32)
    sel[0, 0:64] = 1.0
    sel[1, 64:128] = 1.0

    tags_ext = np.concatenate(
        [np.full((1, B), START, tags.dtype), tags], axis=0)
    emit = np.take_along_axis(
        feats, tags_ext[1:][:, :, None].astype(np.int64), axis=2)[..., 0]
    trg = trans[tags_ext[1:], tags_ext[:-1]]
    endt = trans[END, tags[-1]]
    gb = np.zeros((128, 9, NCORE, 128), np.float32)
    gb[:, 0:4] = emit.reshape(4, 128, NCORE, 128).transpose(1, 0, 2, 3)
    gb[:, 4:8] = trg.reshape(4, 128, NCORE, 128).transpose(1, 0, 2, 3)
    gb[0, 8] = endt.reshape(NCORE, 128)
    GOLD = np.ascontiguousarray(
        gb.transpose(2, 0, 1, 3)).reshape(NCORE, 128, 9 * 128).astype(BF16)
    return FD, cb, sel, GOLD


def _install_ntff_hook():
    """Provide antenv.axon_hooks (absent in this image) so trace=True can
    capture NTFF profiles via the axon .so C ABI."""
    import sys, types, ctypes, contextlib
    if "antenv.axon_hooks" in sys.modules:
        return
    so_path = None
    for line in open("/proc/self/maps"):
        if "libaxon_pjrt.so" in line:
            so_path = line.split()[-1]
            break
    mod = types.ModuleType("antenv.axon_hooks")
    state = {"hook": None}
    if so_path:
        lib = ctypes.CDLL(so_path)
        if hasattr(lib, "axon_start_nrt_profile"):
            lib.axon_start_nrt_profile.argtypes = [
                ctypes.POINTER(ctypes.c_int64), ctypes.c_size_t]
            lib.axon_start_nrt_profile.restype = ctypes.c_int64
            lib.axon_stop_nrt_profile.argtypes = [ctypes.c_char_p]
            lib.axon_stop_nrt_profile.restype = ctypes.c_int64

            @contextlib.contextmanager
            def _hook(output_dir, device_ids):
                import jax
                jax.devices()
                if device_ids:
                    ids = (ctypes.c_int64 * len(device_ids))(*device_ids)
                    rc = lib.axon_start_nrt_profile(ids, len(device_ids))
                else:
                    rc = lib.axon_start_nrt_profile(None, 0)
                if rc != 0:
                    raise RuntimeError(f"axon_start_nrt_profile rc={rc}")
                try:
                    yield
                finally:
                    n = lib.axon_stop_nrt_profile(str(output_dir).encode())
                    print(f"ntff profile: {n} file(s) -> {output_dir}")

            state["hook"] = _hook
    mod.get_axon_ntff_profile_hook = lambda: state["hook"]
    mod.set_axon_ntff_profile_hook = lambda h: state.update(hook=h)
    sys.modules["antenv.axon_hooks"] = mod


def kernel(feats, tags, mask, transition):
    from concourse.bass_utils import run_bass_kernel_spmd
    if os.environ.get("CRF_TRACE", "0") == "1":
        _install_ntff_hook()

    tags_np = np.asarray(tags)
    FD, cb, sel, GOLD = host_prepare(feats, tags_np, transition)
    nc = build()
    in_maps = []
    for c in range(NCORE):
        in_maps.append({"fd": FD[c], "cb": cb, "sel": sel, "gold": GOLD[c]})
    res = run_bass_kernel_spmd(nc, in_maps, list(range(NCORE)),
                               trace=bool(int(os.environ.get("CRF_TRACE", "0"))))
    out = np.concatenate([np.asarray(res.results[c]["out"]).reshape(128)
                          for c in range(NCORE)])
    if getattr(res, "exec_time_ns", None):
        print(f"HW exec time: {res.exec_time_ns} ns")
    return out.astype(np.float32)
